# revision 1
# baseline (speedup 1.0000x reference)
"""AdaZero encoder layer on 8 Trainium2 NeuronCores.

Sharding: zero-collective hybrid. Core c handles batch b = c // 2 and
query-row half h = c % 2 (512 of the 1024 sequence rows). Each core
computes the full K/V for its batch (duplicated across the 2 cores of a
batch, ~14% extra FLOPs) and attention + FFN for its own 512 query rows,
so no inter-core communication is needed. The graph is SPMD-identical
across cores: per-core differences are pushed into the data by rolling
the sequence axis on the host and passing rolled RoPE tables.

Compute dtype: bf16 matmuls with fp32 PSUM accumulation; layernorm stats
and the residual stream stay fp32. The adaLN gates scale the sublayer
outputs by ~0.02, so bf16 error on the sublayer path is ~1e-4 relative
on the final output.
"""

import os
import sys
import types

import numpy as np
import ml_dtypes

D_MODEL = 1024
HEADS = 16
HEAD_DIM = 64
D_FF = 4096
GAMMA_SCALE = 1.0
LN_EPS = 1e-5
ROPE_BASE = 10000.0
B = 4
S = 1024
SH = 512  # query rows per core
N_CORES = 8

_BF16 = ml_dtypes.bfloat16

_graph_cache = {}


def _install_ntff_shim():
    """run_bass_kernel_spmd(trace=True) under axon needs antenv.axon_hooks;
    this image's antenv lacks it, but the ctypes impl lives in trn_agent_boot."""
    if "antenv.axon_hooks" in sys.modules:
        return
    import antenv
    mod = types.ModuleType("antenv.axon_hooks")
    store = {"h": None}
    mod.set_axon_ntff_profile_hook = lambda h: store.__setitem__("h", h)
    mod.get_axon_ntff_profile_hook = lambda: store["h"]
    sys.modules["antenv.axon_hooks"] = mod
    antenv.axon_hooks = mod
    try:
        from trn_agent_boot.trn_boot import _ntff_profile_via_ctypes
        hook = _ntff_profile_via_ctypes("/opt/axon/libaxon_pjrt.so")
        if hook is not None:
            mod.set_axon_ntff_profile_hook(hook)
    except Exception:
        pass


def _build_graph(flags):
    """Build the SPMD per-core Bass graph. `flags` = (mask_trivial, bq_nz,
    bk_nz, bv_nz, bo_nz, b1_nz, b2_nz): whether the mask is all-True and
    whether the (normally all-zero) bias paths are emitted."""
    import concourse.bass as bass
    import concourse.mybir as mybir
    import concourse.tile as tile
    from concourse import bacc
    from concourse.masks import make_identity
    from contextlib import ExitStack

    mtriv, bq_nz, bk_nz, bv_nz, bo_nz, b1_nz, b2_nz = flags
    fp32 = mybir.dt.float32
    bf16 = mybir.dt.bfloat16
    AF = mybir.ActivationFunctionType
    OP = mybir.AluOpType

    nc = bacc.Bacc(None, target_bir_lowering=False)

    # ---- DRAM parameters (per-core shards; all cores share shapes) ----
    x_d = nc.dram_tensor("x", [S, D_MODEL], fp32, kind="ExternalInput")
    wq_d = nc.dram_tensor("wq", [8, 128, 1024], bf16, kind="ExternalInput")  # lhsT-tiled
    wk_d = nc.dram_tensor("wk", [8, 128, 1024], bf16, kind="ExternalInput")  # lhsT-tiled
    wv_d = nc.dram_tensor("wv", [8, 128, 1024], bf16, kind="ExternalInput")  # natural rows
    wo_d = nc.dram_tensor("wo", [8, 128, 1024], bf16, kind="ExternalInput")  # natural rows
    w1_d = nc.dram_tensor("w1", [32, 128, 1024], bf16, kind="ExternalInput")  # lhsT-tiled
    w2_d = nc.dram_tensor("w2", [32, 128, 1024], bf16, kind="ExternalInput")  # natural rows
    cos_d = nc.dram_tensor("cos2", [128, S], bf16, kind="ExternalInput")
    sin_d = nc.dram_tensor("sin2", [128, S], bf16, kind="ExternalInput")
    mod_d = nc.dram_tensor("mod", [4, D_MODEL], bf16, kind="ExternalInput")
    maskb_d = nc.dram_tensor("maskb", [128, 8], fp32, kind="ExternalInput")
    out_d = nc.dram_tensor("out", [SH, D_MODEL], fp32, kind="ExternalOutput")
    bias_d = {}
    if bq_nz:
        bias_d["bq"] = nc.dram_tensor("bq", [128, 8], fp32, kind="ExternalInput")
    if bk_nz:
        bias_d["bk"] = nc.dram_tensor("bk", [128, 8], fp32, kind="ExternalInput")
    if bv_nz:
        bias_d["bv"] = nc.dram_tensor("bv", [D_MODEL], fp32, kind="ExternalInput")
    if bo_nz:
        bias_d["bo"] = nc.dram_tensor("bo", [D_MODEL], fp32, kind="ExternalInput")
    if b1_nz:
        bias_d["b1"] = nc.dram_tensor("b1", [128, 32], fp32, kind="ExternalInput")
    if b2_nz:
        bias_d["b2"] = nc.dram_tensor("b2", [D_MODEL], fp32, kind="ExternalInput")

    with ExitStack() as ctx:
        tc = ctx.enter_context(tile.TileContext(nc))

        const = ctx.enter_context(tc.tile_pool(name="const", bufs=1))
        ident = const.tile([128, 128], bf16)
        make_identity(nc, ident[:])
        # adaLN gamma/beta modulation vectors, broadcast across partitions
        # (the alpha gates are folded into wo/w2 on the host). Tiles are
        # allocated here; the DMAs are emitted inside phase A interleaved
        # with the x loads so the gpsimd queue serves the LN1 critical path
        # in the right order.
        mods = [const.tile([128, D_MODEL], bf16, tag=f"mod{i}", name=f"mod{i}")
                for i in range(4)]
        m0b, b0b, m1b, b1mb = mods

        def mod_dma(i):
            nc.gpsimd.dma_start(out=mods[i][:],
                                in_=bass.AP(tensor=mod_d, offset=i * D_MODEL,
                                            ap=[[0, 128], [1, D_MODEL]]))

        maskb = const.tile([128, 8], fp32)
        cos2 = const.tile([128, S], bf16)
        sin2 = const.tile([128, S], bf16)
        eps_t = const.tile([128, 1], fp32)
        nc.vector.memset(eps_t[:], LN_EPS)
        ones_k = const.tile([128, 64], bf16)
        nc.vector.memset(ones_k[:], 1.0)
        # Dummy Sqrt so the ACT sqrt-table load lands at the very start of
        # the scalar stream, ahead of the scalar-queue DMAs below (the LN1
        # critical path otherwise waits ~7us for it).
        warm = const.tile([128, 1], fp32)
        nc.scalar.activation(warm[:], eps_t[:], AF.Sqrt)
        bias_sb = {}
        for nm in ("bq", "bk", "b1"):
            if nm in bias_d:
                t = const.tile(list(bias_d[nm].shape), fp32, tag=f"bias_{nm}")
                nc.sync.dma_start(out=t[:], in_=bias_d[nm][:])
                bias_sb[nm] = t
        for nm in ("bv", "bo", "b2"):
            if nm in bias_d:
                t = const.tile([128, D_MODEL], fp32, tag=f"bias_{nm}")
                nc.sync.dma_start(out=t[:], in_=bass.AP(tensor=bias_d[nm], offset=0,
                                                        ap=[[0, 128], [1, D_MODEL]]))
                bias_sb[nm] = t

        x_q = ctx.enter_context(tc.tile_pool(name="xq", bufs=1)).tile(
            [128, 4, D_MODEL], fp32)
        O_sb = ctx.enter_context(tc.tile_pool(name="attnO", bufs=1)).tile(
            [128, 8, SH], bf16)  # O^T concat [d, q]
        x1 = ctx.enter_context(tc.tile_pool(name="x1", bufs=1)).tile(
            [128, 4, D_MODEL], fp32)
        # wo reserved early (scoped A..D via its own stack, closed before the
        # ffn pools open); its loads are emitted after the x loads so they
        # fill the scalar queue during phases B/C when it is otherwise idle.
        from contextlib import ExitStack as _ES
        wo_stack = _ES()
        wo_sb = wo_stack.enter_context(tc.tile_pool(name="wo", bufs=1)).tile(
            [128, 8, 1024], bf16)

        def layernorm_mod(x_t, pool, mbt, bbt, tagsfx):
            """LN over free axis + adaLN modulation; returns bf16 [128, D]."""
            stats = pool.tile([128, 2, 6], fp32, tag="stats" + tagsfx)
            nc.vector.bn_stats(out=stats[:, 0, :], in_=x_t[:, 0:512])
            nc.vector.bn_stats(out=stats[:, 1, :], in_=x_t[:, 512:1024])
            mv = pool.tile([128, 2], fp32, tag="mv" + tagsfx)
            nc.vector.bn_aggr(out=mv[:], in_=stats[:])
            std = pool.tile([128, 1], fp32, tag="std" + tagsfx)
            nc.scalar.activation(std[:], mv[:, 1:2], AF.Sqrt, bias=eps_t[:])
            rstd = pool.tile([128, 1], fp32, tag="rstd" + tagsfx)
            nc.vector.reciprocal(rstd[:], std[:])
            nrm = pool.tile([128, D_MODEL], bf16, tag="nrm" + tagsfx)
            nc.vector.tensor_scalar(out=nrm[:], in0=x_t, scalar1=mv[:, 0:1],
                                    scalar2=rstd[:], op0=OP.subtract, op1=OP.mult)
            t1 = pool.tile([128, D_MODEL], bf16, tag="t1" + tagsfx)
            nc.vector.tensor_mul(t1[:], nrm[:], mbt[:])
            nm_ = pool.tile([128, D_MODEL], bf16, tag="nm" + tagsfx)
            nc.vector.tensor_add(nm_[:], t1[:], bbt[:])
            return nm_

        def rope_apply(dst, src, n, pool):
            # dst, src: [128, n] bf16; rotate-half RoPE with sign-folded tables.
            # The rotate-half partition swap must go through DMA (DVE lanes
            # are partition-locked); spread the 4 slab copies over two queues.
            swp = pool.tile([128, n], bf16, tag="ropeswp")
            for lo, sl in ((0, 32), (32, 0), (64, 96), (96, 64)):
                nc.gpsimd.dma_start(out=swp[lo:lo + 32, :], in_=src[sl:sl + 32, :])
            tcos = pool.tile([128, n], bf16, tag="ropecos")
            nc.vector.tensor_mul(tcos[:], src, cos2[:, 0:n])
            tsin = pool.tile([128, n], bf16, tag="ropesin")
            nc.vector.tensor_mul(tsin[:], swp[:], sin2[:, 0:n])
            nc.vector.tensor_add(dst, tcos[:], tsin[:])

        with tc.tile_pool(name="kqv", bufs=1) as kqvp:
            Qt = kqvp.tile([128, 8, SH], bf16)       # Q~^T: [do, q]
            Kt = kqvp.tile([128, 8, S], bf16)        # K~^T: [do, k]
            Vn = kqvp.tile([128, 8, HEADS, HEAD_DIM], bf16)  # V natural

            with tc.tile_pool(name="n1t", bufs=1) as n1tp:
                n1T = n1tp.tile([128, 8, 1024], bf16)   # n1^T: [d, s]

                # ---------- Phase A: LN1 + modulation + transpose ----------
                # All x DMAs are emitted first (interleaved halves on the
                # sync queue) so tile 0 lands ASAP and the LN chain starts
                # without queuing behind constants.
                with tc.tile_pool(name="xkv", bufs=4) as xkvp, \
                     tc.tile_pool(name="ln1", bufs=2) as ln1p, \
                     tc.tile_pool(name="ln1ps", bufs=2, space="PSUM") as lnps:
                    x_tiles = []
                    for st in range(8):
                        if st < 4:
                            x_t = x_q[:, st, :]
                        else:
                            xkv = xkvp.tile([128, D_MODEL], fp32, tag=f"xkv{st}",
                                            name=f"xkv{st}")
                            x_t = xkv[:]
                        nc.sync.dma_start(out=x_t[:, 0:512],
                                          in_=x_d[st * 128:(st + 1) * 128, 0:512])
                        nc.sync.dma_start(out=x_t[:, 512:1024],
                                          in_=x_d[st * 128:(st + 1) * 128, 512:1024])
                        if st == 0:
                            mod_dma(0)  # m0b/b0b first on the gpsimd queue
                            mod_dma(1)
                        x_tiles.append(x_t)
                    nc.gpsimd.dma_start(out=maskb[:], in_=maskb_d[:])
                    nc.gpsimd.dma_start(out=cos2[:], in_=cos_d[:])
                    nc.gpsimd.dma_start(out=sin2[:], in_=sin_d[:])
                    for i in (2, 3):
                        mod_dma(i)
                    for st in range(8):
                        x_t = x_tiles[st]
                        n1m = layernorm_mod(x_t, ln1p, m0b, b0b, "1")
                        for dt in range(8):
                            tps = lnps.tile([128, 128], bf16, tag="tps")
                            nc.tensor.transpose(tps[:], n1m[:, dt * 128:(dt + 1) * 128],
                                                ident[:])
                            nc.scalar.copy(out=n1T[:, dt, st * 128:(st + 1) * 128],
                                           in_=tps[:])

                # ---------- Phase B: QKV projections (+RoPE) ----------
                with tc.tile_pool(name="wstream", bufs=5) as wsp, \
                     tc.tile_pool(name="wv", bufs=1) as wvp, \
                     tc.tile_pool(name="qkvtmp", bufs=3) as qtp, \
                     tc.tile_pool(name="qkvps", bufs=3, space="PSUM") as qps:
                    # Q^T[do, q], q = first 512 rolled rows
                    for m in range(8):
                        wqt = wsp.tile([128, 1024], bf16, tag="wt")
                        nc.sync.dma_start(out=wqt[:], in_=wq_d[m])
                        ps = qps.tile([128, SH], fp32, tag="projps")
                        for k in range(8):
                            nc.tensor.matmul(ps[:], wqt[:, k * 128:(k + 1) * 128],
                                             n1T[:, k, 0:SH],
                                             start=(k == 0), stop=(k == 7))
                        qtmp = qtp.tile([128, SH], bf16, tag="qtmp")
                        if bq_nz:
                            nc.scalar.activation(qtmp[:], ps[:], AF.Copy,
                                                 bias=bias_sb["bq"][:, m:m + 1])
                        else:
                            nc.scalar.copy(out=qtmp[:], in_=ps[:])
                        rope_apply(Qt[:, m, :], qtmp[:], SH, qtp)

                    # K^T[do, k] over all 1024 rolled rows
                    for m in range(8):
                        wkt = wsp.tile([128, 1024], bf16, tag="wt")
                        nc.sync.dma_start(out=wkt[:], in_=wk_d[m])
                        ktmp = qtp.tile([128, S], bf16, tag="ktmp")
                        for nh in range(2):
                            ps = qps.tile([128, SH], fp32, tag="projps")
                            for k in range(8):
                                nc.tensor.matmul(ps[:], wkt[:, k * 128:(k + 1) * 128],
                                                 n1T[:, k, nh * SH:(nh + 1) * SH],
                                                 start=(k == 0), stop=(k == 7))
                            if bk_nz:
                                nc.scalar.activation(ktmp[:, nh * SH:(nh + 1) * SH],
                                                     ps[:], AF.Copy,
                                                     bias=bias_sb["bk"][:, m:m + 1])
                            else:
                                nc.scalar.copy(out=ktmp[:, nh * SH:(nh + 1) * SH],
                                               in_=ps[:])
                        rope_apply(Kt[:, m, :], ktmp[:], S, qtp)

                    # V natural [s, dv]
                    wv_sb = wvp.tile([128, 8, 1024], bf16)
                    for k in range(8):
                        nc.sync.dma_start(out=wv_sb[:, k, :], in_=wv_d[k])
                    # wo streams behind the QKV weights on the sync queue so
                    # it is resident well before phase D
                    for k in range(8):
                        nc.sync.dma_start(out=wo_sb[:, k, :], in_=wo_d[k])
                    for st in range(8):
                        for nh in range(2):
                            ps = qps.tile([128, SH], fp32, tag="projps")
                            for k in range(8):
                                nc.tensor.matmul(ps[:],
                                                 n1T[:, k, st * 128:(st + 1) * 128],
                                                 wv_sb[:, k, nh * SH:(nh + 1) * SH],
                                                 start=(k == 0), stop=(k == 7))
                            src = ps[:]
                            if bv_nz:
                                vtmp = qtp.tile([128, SH], fp32, tag="vtmp")
                                nc.vector.tensor_add(
                                    vtmp[:], ps[:],
                                    bias_sb["bv"][:, nh * SH:(nh + 1) * SH])
                                src = vtmp[:]
                            nc.vector.tensor_copy(
                                out=Vn[:, st, nh * 8:(nh + 1) * 8, :],
                                in_=src.rearrange("p (h d) -> p h d", d=HEAD_DIM))

            # ---------- Phase C: attention ----------
            # Heads run in even/odd pairs. Per pair: scores for both heads
            # (PE row groups 0:64 / 64:128 work concurrently), exp on 2-bank
            # PSUM tiles, then attnV + denominator MMs packed into single
            # [128,512] PSUM banks (even head rows 0:64, odd head rows 64:128
            # via tile_position) so normalization is ONE wide reciprocal and
            # ONE wide multiply writing O_sb directly.
            with tc.tile_pool(name="pt", bufs=2) as ptp, \
                 tc.tile_pool(name="dn", bufs=2) as dnp, \
                 tc.tile_pool(name="spsp", bufs=2, space="PSUM") as spsp, \
                 tc.tile_pool(name="ovdn", bufs=4, space="PSUM") as ovdnp:
                for pr in range(HEADS // 2):
                    mt = pr
                    PTs = [ptp.tile([128, 8, SH], bf16, tag=f"PT{par}",
                                    name=f"PT_{pr}_{par}") for par in range(2)]
                    for kb2 in range(4):
                        sps2 = [spsp.tile([128, 2, SH], fp32, tag="sps",
                                          name=f"sps_{pr}_{kb2}_{par}")
                                for par in range(2)]
                        for sub in range(2):
                            kb = 2 * kb2 + sub
                            for par in range(2):
                                po = par * 64
                                nc.tensor.matmul(
                                    sps2[par][:, sub, :],
                                    Kt[po:po + 64, mt, kb * 128:(kb + 1) * 128],
                                    Qt[po:po + 64, mt, :])
                        for par in range(2):
                            if mtriv:
                                nc.scalar.activation(
                                    PTs[par][:, 2 * kb2:2 * kb2 + 2, :],
                                    sps2[par][:], AF.Exp,
                                    scale=float(1.0 / np.sqrt(HEAD_DIM)))
                            else:
                                for sub in range(2):
                                    kb = 2 * kb2 + sub
                                    nc.scalar.activation(
                                        PTs[par][:, kb, :], sps2[par][:, sub, :],
                                        AF.Exp, bias=maskb[:, kb:kb + 1],
                                        scale=float(1.0 / np.sqrt(HEAD_DIM)))
                    ov2 = ovdnp.tile([128, SH], fp32, tag="ovdn",
                                     name=f"ov_{pr}")
                    dn2 = ovdnp.tile([128, SH], fp32, tag="ovdn",
                                     name=f"dn_{pr}")
                    for kb in range(8):
                        st = (kb == 0)
                        sp = (kb == 7)
                        nc.tensor.matmul(ov2[0:64, :], Vn[:, kb, 2 * pr, :],
                                         PTs[0][:, kb, :], start=st, stop=sp,
                                         skip_group_check=True)
                        nc.tensor.matmul(ov2[64:128, :], Vn[:, kb, 2 * pr + 1, :],
                                         PTs[1][:, kb, :], start=st, stop=sp,
                                         tile_position=(0, 64),
                                         skip_group_check=True)
                        nc.tensor.matmul(dn2[0:64, :], ones_k[:],
                                         PTs[0][:, kb, :], start=st, stop=sp,
                                         skip_group_check=True)
                        nc.tensor.matmul(dn2[64:128, :], ones_k[:],
                                         PTs[1][:, kb, :], start=st, stop=sp,
                                         tile_position=(0, 64),
                                         skip_group_check=True)
                    rrec = dnp.tile([128, SH], fp32, tag="rrec",
                                    name=f"rrec_{pr}")
                    nc.vector.reciprocal(rrec[:], dn2[:])
                    nc.vector.tensor_mul(O_sb[:, mt, :], ov2[:], rrec[:])

        # ---------- Phase D: output projection + residual ----------
        with tc.tile_pool(name="optmp", bufs=3) as opt, \
             tc.tile_pool(name="ops", bufs=3, space="PSUM") as opsp:
            for qb in range(4):
                for nh in range(2):
                    ps = opsp.tile([128, SH], fp32, tag="ops")
                    for k in range(8):
                        nc.tensor.matmul(ps[:],
                                         O_sb[:, k, qb * 128:(qb + 1) * 128],
                                         wo_sb[:, k, nh * SH:(nh + 1) * SH],
                                         start=(k == 0), stop=(k == 7))
                    sl = slice(nh * SH, (nh + 1) * SH)
                    if bo_nz:
                        t2 = opt.tile([128, SH], fp32, tag="opt2")
                        nc.vector.tensor_add(t2[:], ps[:], bias_sb["bo"][:, sl])
                        nc.vector.tensor_add(x1[:, qb, sl], t2[:], x_q[:, qb, sl])
                    else:
                        nc.vector.tensor_add(x1[:, qb, sl], ps[:], x_q[:, qb, sl])
        wo_stack.close()  # release wo's SBUF before the FFN pools open

        # ---------- Phase E: LN2 + FFN + residual + out ----------
        with tc.tile_pool(name="ffn", bufs=1) as ffnp:
            n2T = ffnp.tile([128, 8, SH], bf16)
            hT = ffnp.tile([128, 32, SH], bf16)
            # first half of w2 preloaded on the (now idle) gpsimd queue
            # during LN2/FFN1 so FFN2 is not DMA-gated.
            w2a = ffnp.tile([128, 16, 1024], bf16)
            for j in range(16):
                nc.gpsimd.dma_start(out=w2a[:, j, :], in_=w2_d[j])

            with tc.tile_pool(name="ln2", bufs=2) as ln2p, \
                 tc.tile_pool(name="ln2ps", bufs=2, space="PSUM") as ln2ps:
                for qb in range(4):
                    n2m = layernorm_mod(x1[:, qb, :], ln2p, m1b, b1mb, "2")
                    for dt in range(8):
                        tps = ln2ps.tile([128, 128], bf16, tag="tps2")
                        nc.tensor.transpose(tps[:], n2m[:, dt * 128:(dt + 1) * 128],
                                            ident[:])
                        nc.scalar.copy(out=n2T[:, dt, qb * 128:(qb + 1) * 128],
                                       in_=tps[:])

            # Second half of w2 streams into persistent tiles during FFN1
            # on the (idle) gpsimd queue so FFN2 runs fully resident.
            w2b = [ffnp.tile([128, 1024], bf16, tag=f"w2b{j}", name=f"w2b{j}")
                   for j in range(16)]
            for j in range(16):
                nc.gpsimd.dma_start(out=w2b[j][:], in_=w2_d[16 + j])

            # FFN1: hT[dff, q] = gelu(w1^T @ n2^T)
            with tc.tile_pool(name="w1s", bufs=6) as w1p, \
                 tc.tile_pool(name="f1ps", bufs=2, space="PSUM") as f1ps:
                for j in range(32):
                    w1t = w1p.tile([128, 1024], bf16, tag="w1t")
                    nc.sync.dma_start(out=w1t[:], in_=w1_d[j])
                    ps = f1ps.tile([128, SH], fp32, tag="f1")
                    for k in range(8):
                        nc.tensor.matmul(ps[:], w1t[:, k * 128:(k + 1) * 128],
                                         n2T[:, k, :], start=(k == 0), stop=(k == 7))
                    if b1_nz:
                        nc.scalar.activation(hT[:, j, :], ps[:], AF.Gelu,
                                             bias=bias_sb["b1"][:, j:j + 1])
                    else:
                        nc.scalar.activation(hT[:, j, :], ps[:], AF.Gelu)

            # FFN2: y[q, do], fully weight-resident (w2a + w2b), run in 3
            # staggered qb-groups so earlier groups' epilogues and output
            # DMAs overlap later groups' matmuls; tail is one qb-block.
            with tc.tile_pool(name="f2ps", bufs=1, space="PSUM") as f2ps, \
                 tc.tile_pool(name="otmp", bufs=2) as otp:
                for qbs in ((0, 1), (2,), (3,)):
                    psl = {(qb, nh): f2ps.tile([128, SH], fp32,
                                               tag=f"f2_{qb}_{nh}",
                                               name=f"f2_{qb}_{nh}")
                           for qb in qbs for nh in range(2)}
                    for j in range(32):
                        w2t = w2a[:, j, :] if j < 16 else w2b[j - 16][:]
                        for qb in qbs:
                            for nh in range(2):
                                nc.tensor.matmul(psl[qb, nh][:],
                                                 hT[:, j, qb * 128:(qb + 1) * 128],
                                                 w2t[:, nh * SH:(nh + 1) * SH],
                                                 start=(j == 0), stop=(j == 31))
                    for qb in qbs:
                        for nh in range(2):
                            sl = slice(nh * SH, (nh + 1) * SH)
                            ps = psl[qb, nh]
                            if b2_nz:
                                t2 = otp.tile([128, SH], fp32, tag="ot2")
                                nc.vector.tensor_add(t2[:], ps[:],
                                                     bias_sb["b2"][:, sl])
                                src = t2[:]
                            else:
                                src = ps[:]
                            yo = otp.tile([128, SH], fp32, tag="yo")
                            nc.vector.tensor_add(yo[:], src, x1[:, qb, sl])
                            eng = (nc.gpsimd, nc.scalar)[(qb * 2 + nh) % 2]
                            eng.dma_start(out=out_d[qb * 128:(qb + 1) * 128, sl],
                                          in_=yo[:])

    nc.compile()
    return nc


def _lhsT_tile(w, nblocks_in, nblocks_out):
    # w: [in, out] -> [nblocks_out, 128, nblocks_in*128] with
    # result[m][p, k*128+c] = w[k*128+p, m*128+c]
    kin = w.shape[0] // nblocks_in
    return np.ascontiguousarray(
        w.reshape(nblocks_in, kin, nblocks_out, w.shape[1] // nblocks_out)
        .transpose(2, 1, 0, 3)
        .reshape(nblocks_out, kin, -1))


def kernel(src_reps, src_mask, compact_style,
           ada0_w, ada0_b, ada1_w, ada1_b,
           wq, bq, wk, bk, wv, bv, wo, bo,
           w1, b1, w2, b2):
    trace = bool(os.environ.get("KERNEL_TRACE"))
    if trace:
        _install_ntff_shim()
    from concourse.bass_utils import run_bass_kernel_spmd

    src_reps = np.asarray(src_reps, np.float32)
    src_mask = np.asarray(src_mask)
    compact_style = np.asarray(compact_style, np.float32)

    # ---- host prep: adaLN styles ----
    def styles(ada_w, ada_b):
        cs = compact_style
        silu = cs * (1.0 / (1.0 + np.exp(-cs)))
        st = silu @ np.asarray(ada_w, np.float32) + np.asarray(ada_b, np.float32)
        g, be, al = st[:, :D_MODEL], st[:, D_MODEL:2 * D_MODEL], st[:, 2 * D_MODEL:]
        return (1.0 + np.tanh(g) * GAMMA_SCALE), be, al

    m0, be0, al0 = styles(ada0_w, ada0_b)
    m1, be1, al1 = styles(ada1_w, ada1_b)

    # ---- host prep: weights (cast + tile). The adaLN alpha gates are
    # folded into wo / w2 per batch (each core owns one batch). ----
    wq_l = _lhsT_tile(np.asarray(wq), 8, 8).astype(_BF16)
    wk_l = _lhsT_tile(np.asarray(wk), 8, 8).astype(_BF16)
    wv_n = np.ascontiguousarray(np.asarray(wv).reshape(8, 128, 1024)).astype(_BF16)
    w1_l = _lhsT_tile(np.asarray(w1), 8, 32).astype(_BF16)
    wo_b = [np.ascontiguousarray(
        (np.asarray(wo, np.float32) * al0[b][None, :]).reshape(8, 128, 1024)
    ).astype(_BF16) for b in range(B)]
    w2_b = [np.ascontiguousarray(
        (np.asarray(w2, np.float32) * al1[b][None, :]).reshape(32, 128, 1024)
    ).astype(_BF16) for b in range(B)]

    flags = (bool(np.all(src_mask)),) + tuple(
        bool(np.any(np.asarray(b) != 0)) for b in (bq, bk, bv, bo, b1, b2))
    if flags not in _graph_cache:
        _graph_cache[flags] = _build_graph(flags)
    nc = _graph_cache[flags]

    # ---- host prep: RoPE tables (per roll offset) ----
    inv_freq = 1.0 / (ROPE_BASE **
                      (np.arange(0, HEAD_DIM, 2, dtype=np.float32) / HEAD_DIM))
    d_in_head = np.arange(64)
    fidx = np.where(d_in_head < 32, d_in_head, d_in_head - 32)
    sign = np.where(d_in_head < 32, -1.0, 1.0).astype(np.float32)

    def rope_tables(roll):
        pos = np.roll(np.arange(S, dtype=np.float32), -roll)
        ang = pos[None, :] * inv_freq[fidx][:, None]  # [64, S]
        c = np.cos(ang).astype(np.float32)
        s_ = (np.sin(ang) * sign[:, None]).astype(np.float32)
        return (np.ascontiguousarray(np.concatenate([c, c], 0)).astype(_BF16),
                np.ascontiguousarray(np.concatenate([s_, s_], 0)).astype(_BF16))

    tables = [rope_tables(0), rope_tables(SH)]

    in_maps = []
    for c in range(N_CORES):
        b, h = c // 2, c % 2
        x_c = np.ascontiguousarray(np.roll(src_reps[b], -h * SH, axis=0))
        mb = np.where(np.roll(src_mask[b], -h * SH), 0.0, -60.0).astype(np.float32)
        mod = np.stack([m0[b], be0[b], m1[b], be1[b]])
        im = {
            "x": x_c, "wq": wq_l, "wk": wk_l, "wv": wv_n, "wo": wo_b[b],
            "w1": w1_l, "w2": w2_b[b],
            "cos2": tables[h][0], "sin2": tables[h][1],
            "mod": np.ascontiguousarray(mod.astype(_BF16)),
            "maskb": np.ascontiguousarray(mb.reshape(8, 128).T),
        }
        if flags[1]:
            im["bq"] = np.ascontiguousarray(np.asarray(bq, np.float32).reshape(8, 128).T)
        if flags[2]:
            im["bk"] = np.ascontiguousarray(np.asarray(bk, np.float32).reshape(8, 128).T)
        if flags[3]:
            im["bv"] = np.asarray(bv, np.float32)
        if flags[4]:
            im["bo"] = np.asarray(bo, np.float32) * al0[b]
        if flags[5]:
            im["b1"] = np.ascontiguousarray(np.asarray(b1, np.float32).reshape(32, 128).T)
        if flags[6]:
            im["b2"] = np.asarray(b2, np.float32) * al1[b]
        in_maps.append(im)

    res = run_bass_kernel_spmd(nc, in_maps, core_ids=list(range(N_CORES)),
                               trace=trace)
    kernel.last_result = res

    out = np.empty((B, S, D_MODEL), np.float32)
    for c in range(N_CORES):
        b, h = c // 2, c % 2
        out[b, h * SH:(h + 1) * SH, :] = res.results[c]["out"]
    return out



# revision 20
# speedup vs baseline: 1.2390x; 1.2390x over previous
"""AdaZero encoder layer on 8 Trainium2 NeuronCores.

Sharding: zero-collective hybrid. Core c handles batch b = c // 2 and
query-row half h = c % 2 (512 of the 1024 sequence rows). Each core
computes the full K/V for its batch and attention + FFN for its own 512
query rows; no inter-core communication. Per-core differences are pushed
into the data by rolling the sequence axis on the host.

Compute dtype: fp8e4 DoubleRow matmuls (2x PE throughput) with fp32 PSUM
accumulation for all projections/FFN/attnV; attention scores stay bf16.
Host-side weight scale-ups keep fp8 operands in range; the inverse
scales ride for free in fused epilogues (exp scale, gelu pre-scale,
scalar_tensor_tensor residual adds, and the ones-vector value for the
softmax denominator). LN statistics and the residual stream stay fp32;
LN rstd uses Newton iterations on DVE (inputs are ~unit variance) so the
ACT engine only ever loads the exp and gelu tables. RoPE's rotate-half
partner is made lane-adjacent by a host-side permutation of the head
dims so the swap is a single DVE stream_shuffle. Emission interleaves
per-m Q/K/scores/exp so softmax exp (the ACT-bound stream) overlaps all
projection matmuls.
"""

import os
import sys
import types

import numpy as np
import ml_dtypes

D_MODEL = 1024
HEADS = 16
HEAD_DIM = 64
D_FF = 4096
GAMMA_SCALE = 1.0
LN_EPS = 1e-5
ROPE_BASE = 10000.0
B = 4
S = 1024
SH = 512  # query rows per core
N_CORES = 8

S_QK = 32.0    # wq/wk fp8 scale-up; absorbed by exp scale
S_V = 32.0     # wv scale-up; cancelled by ones_k = S_V in the denominator
S_O = 4096.0   # (wo*alpha0) scale-up; divided out in the D epilogue
S_1 = 32.0     # w1 scale-up; divided out by the gelu pre-scale
S_2 = 4096.0   # (w2*alpha1) scale-up; divided out in the FFN2 epilogue

_BF16 = ml_dtypes.bfloat16
_FP8 = ml_dtypes.float8_e4m3

_graph_cache = {}


def _install_ntff_shim():
    """run_bass_kernel_spmd(trace=True) under axon needs antenv.axon_hooks;
    this image's antenv lacks it, but the ctypes impl lives in trn_agent_boot."""
    if "antenv.axon_hooks" in sys.modules:
        return
    import antenv
    mod = types.ModuleType("antenv.axon_hooks")
    store = {"h": None}
    mod.set_axon_ntff_profile_hook = lambda h: store.__setitem__("h", h)
    mod.get_axon_ntff_profile_hook = lambda: store["h"]
    sys.modules["antenv.axon_hooks"] = mod
    antenv.axon_hooks = mod
    try:
        from trn_agent_boot.trn_boot import _ntff_profile_via_ctypes
        hook = _ntff_profile_via_ctypes("/opt/axon/libaxon_pjrt.so")
        if hook is not None:
            mod.set_axon_ntff_profile_hook(hook)
    except Exception:
        pass


# stream_shuffle mask swapping adjacent lanes within each 32-lane quadrant
_SWAP_MASK = [i ^ 1 for i in range(32)]


def _build_graph(flags):
    import concourse.bass as bass
    import concourse.mybir as mybir
    import concourse.tile as tile
    from concourse import bacc
    from concourse.masks import make_identity
    from contextlib import ExitStack

    mtriv, bq_nz, bk_nz, bv_nz, bo_nz, b1_nz, b2_nz = flags
    fp32 = mybir.dt.float32
    bf16 = mybir.dt.bfloat16
    fp8 = mybir.dt.float8e4
    AF = mybir.ActivationFunctionType
    OP = mybir.AluOpType
    DR = mybir.MatmulPerfMode.DoubleRow

    nc = bacc.Bacc(None, target_bir_lowering=False)

    # ---- DRAM parameters (per-core shards) ----
    x_d = nc.dram_tensor("x", [4, 128, D_MODEL], fp32, kind="ExternalInput")
    xkv_d = nc.dram_tensor("xkv", [4, 128, D_MODEL], bf16, kind="ExternalInput")
    wq_d = nc.dram_tensor("wq", [8, 128, 1024], fp8, kind="ExternalInput")   # lhsT
    wk_d = nc.dram_tensor("wk", [8, 128, 1024], fp8, kind="ExternalInput")   # lhsT
    wv_d = nc.dram_tensor("wv", [8, 128, 1024], fp8, kind="ExternalInput")   # natural
    wo_d = nc.dram_tensor("wo", [8, 128, 1024], fp8, kind="ExternalInput")   # natural
    w1_d = nc.dram_tensor("w1", [32, 128, 1024], fp8, kind="ExternalInput")  # lhsT
    w2_d = nc.dram_tensor("w2", [32, 128, 1024], fp8, kind="ExternalInput")  # natural
    cos_d = nc.dram_tensor("cos2", [128, S], bf16, kind="ExternalInput")
    sin_d = nc.dram_tensor("sin2", [128, S], bf16, kind="ExternalInput")
    mod_d = nc.dram_tensor("mod", [4, 128, 8], fp32, kind="ExternalInput")
    out_d = nc.dram_tensor("out", [SH, D_MODEL], fp32, kind="ExternalOutput")
    bias_d = {}
    if not mtriv:
        bias_d["maskb"] = nc.dram_tensor("maskb", [128, 8], fp32, kind="ExternalInput")
    if bq_nz:
        bias_d["bq"] = nc.dram_tensor("bq", [128, 8], fp32, kind="ExternalInput")
    if bk_nz:
        bias_d["bk"] = nc.dram_tensor("bk", [128, 8], fp32, kind="ExternalInput")
    if bv_nz:
        bias_d["bv"] = nc.dram_tensor("bv", [D_MODEL], fp32, kind="ExternalInput")
    if bo_nz:
        bias_d["bo"] = nc.dram_tensor("bo", [D_MODEL], fp32, kind="ExternalInput")
    if b1_nz:
        bias_d["b1"] = nc.dram_tensor("b1", [128, 32], fp32, kind="ExternalInput")
    if b2_nz:
        bias_d["b2"] = nc.dram_tensor("b2", [D_MODEL], fp32, kind="ExternalInput")
    dbg = bool(os.environ.get("KDBG"))
    dbg_d = {}
    if dbg:
        for nm, shp, dt in (("d_n1T", [128, 8, 1024], fp8),
                            ("d_qt", [128, 8, SH], bf16),
                            ("d_kt", [128, 8, S], bf16),
                            ("d_vn", [128, 8, HEADS, 128], fp8),
                            ("d_pt0", [128, 8, SH], fp8),
                            ("d_pt1", [128, 8, SH], fp8),
                            ("d_osb", [128, 8, SH], fp8),
                            ("d_x1", [128, 4, D_MODEL], fp32),
                            ("d_n2T", [128, 8, SH], fp8),
                            ("d_hT", [128, 32, SH], fp8)):
            dbg_d[nm] = nc.dram_tensor(nm, shp, dt, kind="ExternalOutput")

    with ExitStack() as ctx:
        tc = ctx.enter_context(tile.TileContext(nc))

        const = ctx.enter_context(tc.tile_pool(name="const", bufs=1))
        ident = const.tile([128, 128], bf16)
        make_identity(nc, ident[:])
        mods = [const.tile([128, 8], fp32, tag=f"mod{i}", name=f"mod{i}")
                for i in range(4)]
        m0c, b0c, m1c, b1c = mods
        cos2 = const.tile([128, S], bf16)
        sin2 = const.tile([128, S], bf16)
        eps_t = const.tile([128, 1], fp32)
        nc.vector.memset(eps_t[:], LN_EPS)
        wrm = const.tile([128, 512], bf16)
        nc.vector.memset(wrm[:], 0.001)
        # preload the exp activation table during phase A
        dummy = const.tile([128, 1], fp32)
        nc.scalar.activation(dummy[:], eps_t[:], AF.Exp)

        bias_sb = {}
        for nm in ("maskb", "bq", "bk", "b1"):
            if nm in bias_d:
                t = const.tile(list(bias_d[nm].shape), fp32, tag=f"bias_{nm}")
                nc.gpsimd.dma_start(out=t[:], in_=bias_d[nm][:])
                bias_sb[nm] = t
        for nm in ("bv", "bo", "b2"):
            if nm in bias_d:
                t = const.tile([128, D_MODEL], fp32, tag=f"bias_{nm}")
                nc.gpsimd.dma_start(out=t[:], in_=bass.AP(tensor=bias_d[nm], offset=0,
                                                          ap=[[0, 128], [1, D_MODEL]]))
                bias_sb[nm] = t

        x_q = ctx.enter_context(tc.tile_pool(name="xq", bufs=1)).tile(
            [128, 4, D_MODEL], fp32)
        x1 = ctx.enter_context(tc.tile_pool(name="x1", bufs=1)).tile(
            [128, 4, D_MODEL], fp32)
        O_sb = ctx.enter_context(tc.tile_pool(name="attnO", bufs=1)).tile(
            [128, 8, SH], fp8)
        wo_sb = ctx.enter_context(tc.tile_pool(name="wo", bufs=1)).tile(
            [128, 8, 1024], fp8)

        # ---------- PE warmup: get HAM to K=8/8 before real matmuls ----------
        with tc.tile_pool(name="warm", bufs=1, space="PSUM") as wps:
            wt = wps.tile([128, 512], fp32)
            for _ in range(8):
                nc.tensor.matmul(wt[:], ident[:], wrm[:], start=True, stop=True)

        def rsqrt_batch(pool, var_ap, n, tagsfx):
            """rstd [128, n] = 1/sqrt(var + eps) via Newton steps from seed
            1.0 (inputs are ~unit variance by construction)."""
            ve = pool.tile([128, n], fp32, tag="ve" + tagsfx)
            nc.vector.tensor_scalar_add(ve[:], var_ap, LN_EPS)
            y = pool.tile([128, n], fp32, tag="y" + tagsfx)
            # y1 = 1.5 - 0.5*ve  (exact first NR step from y0=1)
            nc.vector.tensor_scalar(out=y[:], in0=ve[:], scalar1=-0.5, scalar2=1.5,
                                    op0=OP.mult, op1=OP.add)
            t = pool.tile([128, n], fp32, tag="t" + tagsfx)
            u = pool.tile([128, n], fp32, tag="u" + tagsfx)
            for _ in range(2):
                nc.vector.tensor_mul(t[:], y[:], y[:])
                nc.vector.tensor_mul(u[:], t[:], ve[:])
                nc.vector.tensor_scalar(out=u[:], in0=u[:], scalar1=-0.5, scalar2=1.5,
                                        op0=OP.mult, op1=OP.add)
                nc.vector.tensor_mul(y[:], y[:], u[:])
            return y

        def ln_tiles(x_tiles, pool, psp, mc, bc, dst, dst_col, tagsfx):
            """LN over free axis for a group of [128, 1024] tiles; writes
            modulated transposed fp8 output into dst[:, dt, dst_col(ti)]."""
            n = len(x_tiles)
            mv = pool.tile([128, n, 2], fp32, tag="mv" + tagsfx)
            for ti, x_t in enumerate(x_tiles):
                stats = pool.tile([128, 2, 6], fp32, tag="stats" + tagsfx)
                nc.vector.bn_stats(out=stats[:, 0, :], in_=x_t[:, 0:512])
                nc.vector.bn_stats(out=stats[:, 1, :], in_=x_t[:, 512:1024])
                nc.vector.bn_aggr(out=mv[:, ti, :], in_=stats[:])
            rstd = rsqrt_batch(pool, mv[:, :, 1], n, tagsfx)
            for ti, x_t in enumerate(x_tiles):
                nrm = pool.tile([128, D_MODEL], bf16, tag="nrm" + tagsfx)
                nc.vector.tensor_scalar(out=nrm[:], in0=x_t,
                                        scalar1=mv[:, ti, 0:1],
                                        scalar2=rstd[:, ti:ti + 1],
                                        op0=OP.subtract, op1=OP.mult)
                for dt in range(8):
                    tps = psp.tile([128, 128], bf16, tag="tps" + tagsfx)
                    nc.tensor.transpose(tps[:], nrm[:, dt * 128:(dt + 1) * 128],
                                        ident[:])
                    # fused adaLN modulation: gamma/beta are per-partition here
                    nc.vector.tensor_scalar(out=dst[:, dt, dst_col(ti)],
                                            in0=tps[:],
                                            scalar1=mc[:, dt:dt + 1],
                                            scalar2=bc[:, dt:dt + 1],
                                            op0=OP.mult, op1=OP.add)

        def rope_apply(dst, ps_ap, n, pool, eng2, bias_col):
            """dst[128, n] bf16 = rope(ps) straight out of PSUM. The rotate
            partner is lane-adjacent (host permuted), so the swap is one
            stream_shuffle. eng2 runs the sin-mul and add (gpsimd offload
            for K; operands of eng2 ops are SBUF-only)."""
            src = ps_ap
            if bias_col is not None:
                tb = pool.tile([128, n], fp32, tag="ropebias")
                nc.vector.tensor_scalar_add(tb[:], ps_ap, bias_col)
                src = tb[:]
            swp = pool.tile([128, n], fp32, tag="ropeswp")
            nc.vector.stream_shuffle(swp[:], src, _SWAP_MASK)
            tcos = pool.tile([128, n], bf16, tag="ropecos")
            nc.vector.tensor_mul(tcos[:], src, cos2[:, 0:n])
            tsin = pool.tile([128, n], bf16, tag="ropesin")
            eng2.tensor_mul(tsin[:], swp[:], sin2[:, 0:n])
            eng2.tensor_add(dst, tcos[:], tsin[:])

        exp_scale = float(1.0 / (S_QK * S_QK * np.sqrt(HEAD_DIM)))

        with tc.tile_pool(name="bc_sbuf", bufs=1) as bcp, \
             tc.tile_pool(name="ptp", bufs=8) as ptp:
            n1T = bcp.tile([128, 8, 1024], fp8)
            Qt = bcp.tile([128, 8, SH], bf16)
            Kt = bcp.tile([128, 8, S], bf16)
            # per head: cols 0:64 = V (x32), cols 64:128 = 1.0 so each attnV
            # matmul also produces the softmax denominator in rows 64:128.
            # The net /32 from V's scale-up is folded into the D epilogue,
            # keeping O_sb ~0.6 magnitude (fp8-friendly).
            Vn = bcp.tile([128, 8, HEADS, 128], fp8)
            nc.vector.memset(Vn[:, :, :, 64:128], 1.0)
            wv_sb = bcp.tile([128, 8, 1024], fp8)

            # ---------- Phase A: x loads + LN1 (st 0-3 first) ----------
            with tc.tile_pool(name="xkv", bufs=4) as xkvp, \
                 tc.tile_pool(name="ln1", bufs=2) as ln1p, \
                 tc.tile_pool(name="ln1ps", bufs=2, space="PSUM") as lnps:
                for st in range(4):
                    eng = (nc.sync, nc.scalar)[st % 2]
                    eng.dma_start(out=x_q[:, st, :], in_=x_d[st])
                kv_tiles = []
                for st in range(4):
                    xkv = xkvp.tile([128, D_MODEL], bf16, tag=f"xkv{st}",
                                    name=f"xkv{st}")
                    eng = (nc.sync, nc.scalar)[st % 2]
                    eng.dma_start(out=xkv[:], in_=xkv_d[st])
                    kv_tiles.append(xkv)
                nc.gpsimd.dma_start(out=cos2[:], in_=cos_d[:])
                nc.gpsimd.dma_start(out=sin2[:], in_=sin_d[:])
                for i in range(4):
                    nc.gpsimd.dma_start(out=mods[i][:], in_=mod_d[i])

                ln_tiles([x_q[:, st, :] for st in range(4)], ln1p, lnps,
                         m0c, b0c, n1T, lambda ti: slice(ti * 128, ti * 128 + 128),
                         "1a")

                # wv early on gpsimd (needed by phase V)
                for k in range(8):
                    nc.gpsimd.dma_start(out=wv_sb[:, k, :], in_=wv_d[k])

                ln_tiles([t[:] for t in kv_tiles], ln1p, lnps,
                         m0c, b0c, n1T,
                         lambda ti: slice(512 + ti * 128, 512 + ti * 128 + 128),
                         "1a")

            # ---------- Phase B/C ----------
            with tc.tile_pool(name="wstream", bufs=2) as wsp, \
                 tc.tile_pool(name="ropet", bufs=2) as rtp, \
                 tc.tile_pool(name="spsp", bufs=2, space="PSUM") as spsp:

                PTs = {}
                with tc.tile_pool(name="projps", bufs=2, space="PSUM") as qps:
                    for m in range(8):
                        # Q^T[do(m), q]
                        wqt = wsp.tile([128, 8, 128], fp8, tag="wt")
                        nc.sync.dma_start(
                            out=wqt[:],
                            in_=wq_d[m].rearrange("p (a b) -> p a b", b=128))
                        ps = qps.tile([128, SH], fp32, tag="projps")
                        for kp in range(4):
                            nc.tensor.matmul(ps[:], wqt[:, 2 * kp:2 * kp + 2, :],
                                             n1T[:, 2 * kp:2 * kp + 2, 0:SH],
                                             start=(kp == 0), stop=(kp == 3),
                                             perf_mode=DR)
                        rope_apply(Qt[:, m, :], ps[:], SH, rtp, nc.vector,
                                   bias_sb["bq"][:, m:m + 1] if bq_nz else None)

                        # K^T[do(m), k] over all 1024 rows
                        wkt = wsp.tile([128, 8, 128], fp8, tag="wt")
                        nc.sync.dma_start(
                            out=wkt[:],
                            in_=wk_d[m].rearrange("p (a b) -> p a b", b=128))
                        for nh in range(2):
                            ps = qps.tile([128, SH], fp32, tag="projps")
                            for kp in range(4):
                                nc.tensor.matmul(ps[:], wkt[:, 2 * kp:2 * kp + 2, :],
                                                 n1T[:, 2 * kp:2 * kp + 2,
                                                     nh * SH:(nh + 1) * SH],
                                                 start=(kp == 0), stop=(kp == 3),
                                                 perf_mode=DR)
                            rope_apply(Kt[:, m, nh * SH:(nh + 1) * SH], ps[:], SH,
                                       rtp, nc.gpsimd,
                                       bias_sb["bk"][:, m:m + 1] if bk_nz else None)

                        # scores + exp for head pair m (even head rows 0:64,
                        # odd head rows 64:128 on separate PE row groups)
                        PT = [ptp.tile([128, 8, SH], fp8, tag=f"PT{par}",
                                       name=f"PT_{m}_{par}") for par in range(2)]
                        PTs[m] = PT
                        for kb2 in range(4):
                            sps2 = [spsp.tile([128, 2, SH], fp32, tag="sps",
                                              name=f"sps_{m}_{kb2}_{par}")
                                    for par in range(2)]
                            for sub in range(2):
                                kb = 2 * kb2 + sub
                                for par in range(2):
                                    po = par * 64
                                    nc.tensor.matmul(
                                        sps2[par][:, sub, :],
                                        Kt[po:po + 64, m, kb * 128:(kb + 1) * 128],
                                        Qt[po:po + 64, m, :])
                            for par in range(2):
                                if mtriv:
                                    nc.scalar.activation(
                                        PT[par][:, 2 * kb2:2 * kb2 + 2, :],
                                        sps2[par][:], AF.Exp, scale=exp_scale)
                                else:
                                    for sub in range(2):
                                        kb = 2 * kb2 + sub
                                        nc.scalar.activation(
                                            PT[par][:, kb, :],
                                            sps2[par][:, sub, :], AF.Exp,
                                            bias=bias_sb["maskb"][:, kb:kb + 1],
                                            scale=exp_scale)

                    # V natural [s, dv]
                    for st in range(8):
                        for nh in range(2):
                            ps = qps.tile([128, SH], fp32, tag="projps")
                            for kp in range(4):
                                nc.tensor.matmul(
                                    ps[:],
                                    n1T[:, 2 * kp:2 * kp + 2,
                                        st * 128:(st + 1) * 128],
                                    wv_sb[:, 2 * kp:2 * kp + 2,
                                          nh * SH:(nh + 1) * SH],
                                    start=(kp == 0), stop=(kp == 3), perf_mode=DR)
                            src = ps[:]
                            if bv_nz:
                                vtmp = rtp.tile([128, SH], fp32, tag="vtmp")
                                nc.vector.tensor_add(
                                    vtmp[:], ps[:],
                                    bias_sb["bv"][:, nh * SH:(nh + 1) * SH])
                                src = vtmp[:]
                            nc.vector.tensor_copy(
                                out=Vn[:, st, nh * 8:(nh + 1) * 8, 0:64],
                                in_=src.rearrange("p (h d) -> p h d", d=HEAD_DIM))

                    # wo prefetch on the now-idle sync queue
                    for k in range(8):
                        nc.sync.dma_start(out=wo_sb[:, k, :], in_=wo_d[k])
                    if dbg:
                        nc.sync.dma_start(out=dbg_d["d_n1T"][:], in_=n1T[:])
                        nc.sync.dma_start(out=dbg_d["d_qt"][:], in_=Qt[:])
                        nc.sync.dma_start(out=dbg_d["d_kt"][:], in_=Kt[:])
                        nc.sync.dma_start(out=dbg_d["d_vn"][:], in_=Vn[:])
                        nc.sync.dma_start(out=dbg_d["d_pt0"][:], in_=PTs[0][0][:])
                        nc.sync.dma_start(out=dbg_d["d_pt1"][:], in_=PTs[0][1][:])

                # attnV + denominator, normalize. ovdn's banks come from the
                # 2 never-used banks + projps's (spsp stays open so scores
                # banks aren't recycled under the still-draining exp stream).
                with tc.tile_pool(name="ovdn", bufs=4, space="PSUM") as ovp, \
                     tc.tile_pool(name="dnt", bufs=4) as dnp:
                    for m in range(8):
                        PT = PTs[m]
                        for h2 in range(2):
                            h = 2 * m + h2
                            pv = ovp.tile([128, SH], fp32, tag="ovdn",
                                          name=f"ov_{m}_{h2}")
                            for kp in range(4):
                                nc.tensor.matmul(
                                    pv[:], Vn[:, 2 * kp:2 * kp + 2, h, :],
                                    PT[h2][:, 2 * kp:2 * kp + 2, :],
                                    start=(kp == 0), stop=(kp == 3),
                                    perf_mode=DR)
                            # rows 0:64 = P@V*32, rows 64:128 = sum(P); the
                            # reciprocal is shifted down 64 partitions by DMA
                            rr = dnp.tile([128, SH], fp32, tag="rr",
                                          name=f"rr_{m}_{h2}")
                            nc.vector.reciprocal(rr[64:128, :],
                                                 pv[64:128, :])
                            nc.gpsimd.dma_start(out=rr[0:64, :],
                                                in_=rr[64:128, :])
                            if h2 == 0:
                                nc.vector.tensor_mul(O_sb[0:64, m, :],
                                                     pv[0:64, :], rr[0:64, :])
                            else:
                                ot = dnp.tile([128, SH], fp8, tag="ot",
                                              name=f"ot_{m}")
                                nc.vector.tensor_mul(ot[0:64, :],
                                                     pv[0:64, :], rr[0:64, :])
                                nc.gpsimd.dma_start(out=O_sb[64:128, m, :],
                                                    in_=ot[0:64, :])

        # table switch to gelu happens here, hidden behind phase D
        nc.scalar.activation(dummy[:], eps_t[:], AF.Gelu)
        if dbg:
            nc.sync.dma_start(out=dbg_d["d_osb"][:], in_=O_sb[:])

        # ---------- Phase D: output projection + residual; LN2; FFN ----------
        with tc.tile_pool(name="ffn", bufs=1) as ffnp:
            n2T = ffnp.tile([128, 8, SH], fp8)
            hT = ffnp.tile([128, 32, SH], fp8)
            w1a = ffnp.tile([128, 16, 8, 128], fp8)
            w2a = ffnp.tile([128, 32, 1024], fp8)
            for j in range(16):
                nc.sync.dma_start(
                    out=w1a[:, j, :, :],
                    in_=w1_d[j].rearrange("p (a b) -> p a b", b=128))
            for j in range(16):
                nc.gpsimd.dma_start(out=w2a[:, j, :], in_=w2_d[j])

            xr = x_q
            if bo_nz:
                xr = ffnp.tile([128, 4, D_MODEL], fp32, tag="xqb")
                for qb in range(4):
                    nc.vector.tensor_add(xr[:, qb, :], x_q[:, qb, :],
                                         bias_sb["bo"][:])

            with tc.tile_pool(name="ops", bufs=2, space="PSUM") as opsp, \
                 tc.tile_pool(name="ln2", bufs=2) as ln2p, \
                 tc.tile_pool(name="ln2ps", bufs=2, space="PSUM") as lnps2:
                for qb in range(4):
                    for nh in range(2):
                        ps = opsp.tile([128, SH], fp32, tag="ops")
                        for kp in range(4):
                            nc.tensor.matmul(
                                ps[:],
                                O_sb[:, 2 * kp:2 * kp + 2, qb * 128:(qb + 1) * 128],
                                wo_sb[:, 2 * kp:2 * kp + 2, nh * SH:(nh + 1) * SH],
                                start=(kp == 0), stop=(kp == 3), perf_mode=DR)
                        sl = slice(nh * SH, (nh + 1) * SH)
                        # 1/S_V undoes O_sb's deliberate x32 carry
                        nc.vector.scalar_tensor_tensor(
                            out=x1[:, qb, sl], in0=ps[:],
                            scalar=1.0 / (S_O * S_V),
                            in1=xr[:, qb, sl], op0=OP.mult, op1=OP.add)
                # LN2 (all 4 qb) -> n2T
                ln_tiles([x1[:, qb, :] for qb in range(4)], ln2p, lnps2,
                         m1c, b1c, n2T,
                         lambda ti: slice(ti * 128, ti * 128 + 128), "2")

            if dbg:
                nc.sync.dma_start(out=dbg_d["d_x1"][:], in_=x1[:])
                nc.sync.dma_start(out=dbg_d["d_n2T"][:], in_=n2T[:])
            # second half of w2 streams during FFN1
            for j in range(16, 32):
                nc.gpsimd.dma_start(out=w2a[:, j, :], in_=w2_d[j])

            # FFN1: hT[dff, q] = gelu((w1*S1)^T @ n2^T) via gelu pre-scale
            with tc.tile_pool(name="w1s", bufs=4) as w1p, \
                 tc.tile_pool(name="f1ps", bufs=2, space="PSUM") as f1ps:
                for j in range(32):
                    w1tt = None
                    if j >= 16:
                        w1tt = w1p.tile([128, 8, 128], fp8, tag="w1t")
                        nc.sync.dma_start(
                            out=w1tt[:],
                            in_=w1_d[j].rearrange("p (a b) -> p a b", b=128))
                    ps = f1ps.tile([128, SH], fp32, tag="f1")
                    for kp in range(4):
                        lhs = (w1a[:, j, 2 * kp:2 * kp + 2, :] if j < 16
                               else w1tt[:, 2 * kp:2 * kp + 2, :])
                        nc.tensor.matmul(ps[:], lhs,
                                         n2T[:, 2 * kp:2 * kp + 2, :],
                                         start=(kp == 0), stop=(kp == 3),
                                         perf_mode=DR)
                    if b1_nz:
                        nc.scalar.activation(hT[:, j, :], ps[:], AF.Gelu,
                                             bias=bias_sb["b1"][:, j:j + 1],
                                             scale=1.0 / S_1)
                    else:
                        nc.scalar.activation(hT[:, j, :], ps[:], AF.Gelu,
                                             scale=1.0 / S_1)

            xres = x1
            if b2_nz:
                xres = ffnp.tile([128, 4, D_MODEL], fp32, tag="xres")
                for qb in range(4):
                    nc.vector.tensor_add(xres[:, qb, :], x1[:, qb, :],
                                         bias_sb["b2"][:])

            if dbg:
                nc.sync.dma_start(out=dbg_d["d_hT"][:], in_=hT[:])
            # FFN2: per-qb staggered so epilogues/DMAs overlap later matmuls
            with tc.tile_pool(name="f2ps", bufs=4, space="PSUM") as f2ps, \
                 tc.tile_pool(name="otmp", bufs=4) as otp:
                out_engs = (nc.sync, nc.gpsimd, nc.scalar)
                for qb in range(4):
                    psl = [f2ps.tile([128, SH], fp32, tag="f2",
                                     name=f"f2_{qb}_{nh}") for nh in range(2)]
                    for jp in range(16):
                        for nh in range(2):
                            nc.tensor.matmul(
                                psl[nh][:],
                                hT[:, 2 * jp:2 * jp + 2, qb * 128:(qb + 1) * 128],
                                w2a[:, 2 * jp:2 * jp + 2, nh * SH:(nh + 1) * SH],
                                start=(jp == 0), stop=(jp == 15), perf_mode=DR)
                    for nh in range(2):
                        sl = slice(nh * SH, (nh + 1) * SH)
                        yo = otp.tile([128, SH], fp32, tag="yo")
                        nc.vector.scalar_tensor_tensor(
                            out=yo[:], in0=psl[nh][:], scalar=1.0 / S_2,
                            in1=xres[:, qb, sl], op0=OP.mult, op1=OP.add)
                        if qb < 3:
                            eng = out_engs[(qb * 2 + nh) % 2]
                            eng.dma_start(out=out_d[qb * 128:(qb + 1) * 128, sl],
                                          in_=yo[:])
                        else:
                            # spread the last tiles across 4 queues
                            for q4 in range(2):
                                eng = out_engs[(nh * 2 + q4) % 3]
                                s2 = slice(nh * SH + q4 * 256,
                                           nh * SH + q4 * 256 + 256)
                                eng.dma_start(
                                    out=out_d[qb * 128:(qb + 1) * 128, s2],
                                    in_=yo[:, q4 * 256:q4 * 256 + 256])

    nc.compile()
    return nc


def _lhsT_tile(w, nblocks_in, nblocks_out):
    # w: [in, out] -> [nblocks_out, 128, nblocks_in*128] with
    # result[m][p, k*128+c] = w[k*128+p, m*128+c]
    kin = w.shape[0] // nblocks_in
    return np.ascontiguousarray(
        w.reshape(nblocks_in, kin, nblocks_out, w.shape[1] // nblocks_out)
        .transpose(2, 1, 0, 3)
        .reshape(nblocks_out, kin, -1))


def _fp8(a):
    return np.clip(np.asarray(a, np.float32), -240.0, 240.0).astype(_FP8)


def kernel(src_reps, src_mask, compact_style,
           ada0_w, ada0_b, ada1_w, ada1_b,
           wq, bq, wk, bk, wv, bv, wo, bo,
           w1, b1, w2, b2):
    trace = bool(os.environ.get("KERNEL_TRACE"))
    if trace:
        _install_ntff_shim()
    from concourse.bass_utils import run_bass_kernel_spmd

    src_reps = np.asarray(src_reps, np.float32)
    src_mask = np.asarray(src_mask)
    compact_style = np.asarray(compact_style, np.float32)

    # ---- host prep: adaLN styles ----
    def styles(ada_w, ada_b):
        cs = compact_style
        silu = cs * (1.0 / (1.0 + np.exp(-cs)))
        st = silu @ np.asarray(ada_w, np.float32) + np.asarray(ada_b, np.float32)
        g, be, al = st[:, :D_MODEL], st[:, D_MODEL:2 * D_MODEL], st[:, 2 * D_MODEL:]
        return (1.0 + np.tanh(g) * GAMMA_SCALE), be, al

    m0, be0, al0 = styles(ada0_w, ada0_b)
    m1, be1, al1 = styles(ada1_w, ada1_b)

    # ---- host prep: RoPE head-dim interleave permutation ----
    # new position j within a head holds original dim (j//2) if j even else
    # (j//2 + 32); the rotate partner is then the adjacent lane.
    j = np.arange(HEAD_DIM)
    perm = np.where(j % 2 == 0, j // 2, j // 2 + 32)
    perm_full = (np.arange(D_MODEL) // HEAD_DIM) * HEAD_DIM + \
        np.tile(perm, HEADS)

    # ---- host prep: weights (permute + scale + cast + tile) ----
    wq_p = np.asarray(wq, np.float32)[:, perm_full] * S_QK
    wk_p = np.asarray(wk, np.float32)[:, perm_full] * S_QK
    wq_l = _fp8(_lhsT_tile(wq_p, 8, 8))
    wk_l = _fp8(_lhsT_tile(wk_p, 8, 8))
    wv_n = _fp8((np.asarray(wv, np.float32) * S_V).reshape(8, 128, 1024))
    w1_l = _fp8(_lhsT_tile(np.asarray(w1, np.float32) * S_1, 8, 32))
    wo_b = [_fp8(((np.asarray(wo, np.float32) * al0[b][None, :]) * S_O)
                 .reshape(8, 128, 1024)) for b in range(B)]
    w2_b = [_fp8(((np.asarray(w2, np.float32) * al1[b][None, :]) * S_2)
                 .reshape(32, 128, 1024)) for b in range(B)]

    flags = (bool(np.all(src_mask)),) + tuple(
        bool(np.any(np.asarray(b) != 0)) for b in (bq, bk, bv, bo, b1, b2))
    if flags not in _graph_cache:
        _graph_cache[flags] = _build_graph(flags)
    nc = _graph_cache[flags]

    # ---- host prep: RoPE tables (permuted rows, sign folded into sin) ----
    inv_freq = 1.0 / (ROPE_BASE **
                      (np.arange(0, HEAD_DIM, 2, dtype=np.float32) / HEAD_DIM))
    # at permuted position j: freq index = j//2, sign = -1 for even j
    fidx = np.arange(HEAD_DIM) // 2
    sign = np.where(np.arange(HEAD_DIM) % 2 == 0, -1.0, 1.0).astype(np.float32)

    def rope_tables(roll):
        pos = np.roll(np.arange(S, dtype=np.float32), -roll)
        ang = pos[None, :] * inv_freq[fidx][:, None]  # [64, S]
        c = np.cos(ang).astype(np.float32)
        s_ = (np.sin(ang) * sign[:, None]).astype(np.float32)
        return (np.ascontiguousarray(np.concatenate([c, c], 0)).astype(_BF16),
                np.ascontiguousarray(np.concatenate([s_, s_], 0)).astype(_BF16))

    tables = [rope_tables(0), rope_tables(SH)]

    in_maps = []
    for c in range(N_CORES):
        b, h = c // 2, c % 2
        x_c = np.roll(src_reps[b], -h * SH, axis=0)
        # gamma/beta as per-partition columns: modc[p, k] = mod[k*128+p]
        mod = np.stack([m0[b], be0[b], m1[b], be1[b]])  # [4, 1024]
        modc = np.ascontiguousarray(
            mod.reshape(4, 8, 128).transpose(0, 2, 1).astype(np.float32))
        im = {
            "x": np.ascontiguousarray(x_c[0:512].reshape(4, 128, D_MODEL)),
            "xkv": np.ascontiguousarray(
                x_c[512:1024].reshape(4, 128, D_MODEL)).astype(_BF16),
            "wq": wq_l, "wk": wk_l, "wv": wv_n, "wo": wo_b[b],
            "w1": w1_l, "w2": w2_b[b],
            "cos2": tables[h][0], "sin2": tables[h][1],
            "mod": modc,
        }
        if not flags[0]:
            mb = np.where(np.roll(src_mask[b], -h * SH), 0.0, -60.0)
            im["maskb"] = np.ascontiguousarray(
                mb.reshape(8, 128).T.astype(np.float32))
        if flags[1]:
            im["bq"] = np.ascontiguousarray(
                (np.asarray(bq, np.float32) * S_QK)[perm_full]
                .reshape(8, 128).T)
        if flags[2]:
            im["bk"] = np.ascontiguousarray(
                (np.asarray(bk, np.float32) * S_QK)[perm_full]
                .reshape(8, 128).T)
        if flags[3]:
            im["bv"] = np.asarray(bv, np.float32) * S_V
        if flags[4]:
            im["bo"] = np.asarray(bo, np.float32) * al0[b]
        if flags[5]:
            im["b1"] = np.ascontiguousarray(
                (np.asarray(b1, np.float32) * S_1).reshape(32, 128).T)
        if flags[6]:
            im["b2"] = np.asarray(b2, np.float32) * al1[b]
        in_maps.append(im)

    res = run_bass_kernel_spmd(nc, in_maps, core_ids=list(range(N_CORES)),
                               trace=trace)
    kernel.last_result = res

    out = np.empty((B, S, D_MODEL), np.float32)
    for c in range(N_CORES):
        b, h = c // 2, c % 2
        out[b, h * SH:(h + 1) * SH, :] = res.results[c]["out"]
    return out


# revision 24
# speedup vs baseline: 1.4227x; 1.1482x over previous
"""AdaZero encoder layer on 8 Trainium2 NeuronCores.

Sharding: zero-collective hybrid. Core c handles batch b = c // 2 and
query-row half h = c % 2 (512 of the 1024 sequence rows). Each core
computes the full K/V for its batch and attention + FFN for its own 512
query rows; no inter-core communication. Per-core differences are pushed
into the data by rolling the sequence axis on the host.

Compute dtype: fp8e4 DoubleRow matmuls (2x PE throughput) with fp32 PSUM
accumulation for all projections/FFN/attnV; attention scores stay bf16.
Host-side weight scale-ups keep fp8 operands in range; the inverse
scales ride for free in fused epilogues (exp scale, gelu pre-scale,
scalar_tensor_tensor residual adds, and the ones-vector value for the
softmax denominator). LN statistics and the residual stream stay fp32;
LN rstd uses Newton iterations on DVE (inputs are ~unit variance) so the
ACT engine only ever loads the exp and gelu tables. RoPE's rotate-half
partner is made lane-adjacent by a host-side permutation of the head
dims so the swap is a single DVE stream_shuffle. Emission interleaves
per-m Q/K/scores/exp so softmax exp (the ACT-bound stream) overlaps all
projection matmuls.
"""

import os
import sys
import types

import numpy as np
import ml_dtypes

D_MODEL = 1024
HEADS = 16
HEAD_DIM = 64
D_FF = 4096
GAMMA_SCALE = 1.0
LN_EPS = 1e-5
ROPE_BASE = 10000.0
B = 4
S = 1024
SH = 512  # query rows per core
N_CORES = 8

S_QK = 32.0    # wq/wk fp8 scale-up; absorbed by exp scale
S_V = 32.0     # wv scale-up; cancelled by ones_k = S_V in the denominator
S_O = 4096.0   # (wo*alpha0) scale-up; divided out in the D epilogue
S_1 = 32.0     # w1 scale-up; divided out by the gelu pre-scale
S_2 = 4096.0   # (w2*alpha1) scale-up; divided out in the FFN2 epilogue

_BF16 = ml_dtypes.bfloat16
_FP8 = ml_dtypes.float8_e4m3

_graph_cache = {}


def _install_ntff_shim():
    """run_bass_kernel_spmd(trace=True) under axon needs antenv.axon_hooks;
    this image's antenv lacks it, but the ctypes impl lives in trn_agent_boot."""
    if "antenv.axon_hooks" in sys.modules:
        return
    import antenv
    mod = types.ModuleType("antenv.axon_hooks")
    store = {"h": None}
    mod.set_axon_ntff_profile_hook = lambda h: store.__setitem__("h", h)
    mod.get_axon_ntff_profile_hook = lambda: store["h"]
    sys.modules["antenv.axon_hooks"] = mod
    antenv.axon_hooks = mod
    try:
        from trn_agent_boot.trn_boot import _ntff_profile_via_ctypes
        hook = _ntff_profile_via_ctypes("/opt/axon/libaxon_pjrt.so")
        if hook is not None:
            mod.set_axon_ntff_profile_hook(hook)
    except Exception:
        pass


# stream_shuffle mask swapping adjacent lanes within each 32-lane quadrant
_SWAP_MASK = [i ^ 1 for i in range(32)]


def _build_graph(flags):
    import concourse.bass as bass
    import concourse.mybir as mybir
    import concourse.tile as tile
    from concourse import bacc
    from concourse.masks import make_identity
    from contextlib import ExitStack

    mtriv, bq_nz, bk_nz, bv_nz, bo_nz, b1_nz, b2_nz = flags
    fp32 = mybir.dt.float32
    bf16 = mybir.dt.bfloat16
    fp8 = mybir.dt.float8e4
    AF = mybir.ActivationFunctionType
    OP = mybir.AluOpType
    DR = mybir.MatmulPerfMode.DoubleRow

    nc = bacc.Bacc(None, target_bir_lowering=False)

    # ---- DRAM parameters (per-core shards) ----
    x_d = nc.dram_tensor("x", [4, 128, D_MODEL], fp32, kind="ExternalInput")
    xkv_d = nc.dram_tensor("xkv", [4, 128, D_MODEL], bf16, kind="ExternalInput")
    wq_d = nc.dram_tensor("wq", [8, 128, 1024], fp8, kind="ExternalInput")   # lhsT
    wk_d = nc.dram_tensor("wk", [8, 128, 1024], fp8, kind="ExternalInput")   # lhsT
    wv_d = nc.dram_tensor("wv", [8, 128, 1024], fp8, kind="ExternalInput")   # natural
    wo_d = nc.dram_tensor("wo", [8, 128, 1024], fp8, kind="ExternalInput")   # natural
    w1_d = nc.dram_tensor("w1", [32, 128, 1024], fp8, kind="ExternalInput")  # lhsT
    w2_d = nc.dram_tensor("w2", [32, 128, 1024], fp8, kind="ExternalInput")  # natural
    cos_d = nc.dram_tensor("cos2", [128, S], bf16, kind="ExternalInput")
    sin_d = nc.dram_tensor("sin2", [128, S], bf16, kind="ExternalInput")
    betar_d = nc.dram_tensor("betar", [2, 128, 8, 128], fp8, kind="ExternalInput")
    out_d = nc.dram_tensor("out", [SH, D_MODEL], fp32, kind="ExternalOutput")
    bias_d = {}
    if not mtriv:
        bias_d["maskb"] = nc.dram_tensor("maskb", [128, 8], fp32, kind="ExternalInput")
    if bq_nz:
        bias_d["bq"] = nc.dram_tensor("bq", [128, 8], fp32, kind="ExternalInput")
    if bk_nz:
        bias_d["bk"] = nc.dram_tensor("bk", [128, 8], fp32, kind="ExternalInput")
    if bv_nz:
        bias_d["bv"] = nc.dram_tensor("bv", [D_MODEL], fp32, kind="ExternalInput")
    if bo_nz:
        bias_d["bo"] = nc.dram_tensor("bo", [D_MODEL], fp32, kind="ExternalInput")
    if b1_nz:
        bias_d["b1"] = nc.dram_tensor("b1", [128, 32], fp32, kind="ExternalInput")
    if b2_nz:
        bias_d["b2"] = nc.dram_tensor("b2", [D_MODEL], fp32, kind="ExternalInput")
    dbg = bool(os.environ.get("KDBG"))
    dbg_d = {}
    if dbg:
        for nm, shp, dt in (("d_n1T", [128, 8, 1024], fp8),
                            ("d_qt", [128, 8, SH], bf16),
                            ("d_kt", [128, 8, S], bf16),
                            ("d_vn", [128, 8, HEADS, 128], fp8),
                            ("d_pt0", [128, 8, SH], fp8),
                            ("d_pt1", [128, 8, SH], fp8),
                            ("d_osb", [128, 8, SH], fp8),
                            ("d_x1", [128, 4, D_MODEL], fp32),
                            ("d_n2T", [128, 8, SH], fp8),
                            ("d_hT", [128, 32, SH], fp8)):
            dbg_d[nm] = nc.dram_tensor(nm, shp, dt, kind="ExternalOutput")

    with ExitStack() as ctx:
        tc = ctx.enter_context(tile.TileContext(nc))

        const = ctx.enter_context(tc.tile_pool(name="const", bufs=1))
        ident = const.tile([128, 128], bf16)
        make_identity(nc, ident[:])
        betar = [const.tile([128, 8, 128], fp8, tag=f"betar{i}",
                            name=f"betar{i}") for i in range(2)]
        b0r, b1r = betar
        cos2 = const.tile([128, S], bf16)
        sin2 = const.tile([128, S], bf16)
        eps_t = const.tile([128, 1], fp32)
        nc.vector.memset(eps_t[:], LN_EPS)
        wrm = const.tile([128, 512], fp8)
        nc.vector.memset(wrm[:], 0.001)
        # preload the exp activation table during phase A
        dummy = const.tile([128, 1], fp32)
        nc.scalar.activation(dummy[:], eps_t[:], AF.Exp)

        bias_sb = {}
        for nm in ("maskb", "bq", "bk", "b1"):
            if nm in bias_d:
                t = const.tile(list(bias_d[nm].shape), fp32, tag=f"bias_{nm}")
                nc.gpsimd.dma_start(out=t[:], in_=bias_d[nm][:])
                bias_sb[nm] = t
        for nm in ("bv", "bo", "b2"):
            if nm in bias_d:
                t = const.tile([128, D_MODEL], fp32, tag=f"bias_{nm}")
                nc.gpsimd.dma_start(out=t[:], in_=bass.AP(tensor=bias_d[nm], offset=0,
                                                          ap=[[0, 128], [1, D_MODEL]]))
                bias_sb[nm] = t

        x_q = ctx.enter_context(tc.tile_pool(name="xq", bufs=1)).tile(
            [128, 4, D_MODEL], fp32)
        x1 = ctx.enter_context(tc.tile_pool(name="x1", bufs=1)).tile(
            [128, 4, D_MODEL], fp32)
        O_sb = ctx.enter_context(tc.tile_pool(name="attnO", bufs=1)).tile(
            [128, 8, SH], fp8)
        wo_sb = ctx.enter_context(tc.tile_pool(name="wo", bufs=1)).tile(
            [128, 8, 1024], fp8)

        # ---------- PE warmup: get HAM to K=8/8 before real matmuls ----------
        with tc.tile_pool(name="warm", bufs=1, space="PSUM") as wps:
            wt = wps.tile([128, 512], fp32)
            for _ in range(8):
                nc.tensor.matmul(wt[:], ident[:], wrm[:], start=True, stop=True)

        def rsqrt_batch(pool, var_ap, n, tagsfx):
            """rstd [128, n] = 1/sqrt(var + eps) via Newton steps from seed
            1.0 (inputs are ~unit variance by construction)."""
            ve = pool.tile([128, n], fp32, tag="ve" + tagsfx)
            nc.vector.tensor_scalar_add(ve[:], var_ap, LN_EPS)
            y = pool.tile([128, n], fp32, tag="y" + tagsfx)
            # y1 = 1.5 - 0.5*ve  (exact first NR step from y0=1)
            nc.vector.tensor_scalar(out=y[:], in0=ve[:], scalar1=-0.5, scalar2=1.5,
                                    op0=OP.mult, op1=OP.add)
            t = pool.tile([128, n], fp32, tag="t" + tagsfx)
            u = pool.tile([128, n], fp32, tag="u" + tagsfx)
            for _ in range(2):
                nc.vector.tensor_mul(t[:], y[:], y[:])
                nc.vector.tensor_mul(u[:], t[:], ve[:])
                nc.vector.tensor_scalar(out=u[:], in0=u[:], scalar1=-0.5, scalar2=1.5,
                                        op0=OP.mult, op1=OP.add)
                nc.vector.tensor_mul(y[:], y[:], u[:])
            return y

        def ln_tiles(x_tiles, pool, psp, brep, dst, dst_col, tagsfx):
            """LN over free axis for a group of [128, 1024] tiles. adaLN
            gamma is folded into the projection weights host-side; beta/gamma
            is added here 4-transposes-wide. Writes transposed fp8 output
            into dst[:, dt, dst_col(ti)]."""
            n = len(x_tiles)
            mv = pool.tile([128, n, 2], fp32, tag="mv" + tagsfx)
            for ti, x_t in enumerate(x_tiles):
                stats = pool.tile([128, 2, 6], fp32, tag="stats" + tagsfx)
                nc.vector.bn_stats(out=stats[:, 0, :], in_=x_t[:, 0:512])
                nc.vector.bn_stats(out=stats[:, 1, :], in_=x_t[:, 512:1024])
                nc.vector.bn_aggr(out=mv[:, ti, :], in_=stats[:])
            rstd = rsqrt_batch(pool, mv[:, :, 1], n, tagsfx)
            for ti, x_t in enumerate(x_tiles):
                nrm = pool.tile([128, D_MODEL], bf16, tag="nrm" + tagsfx)
                nc.vector.tensor_scalar(out=nrm[:], in0=x_t,
                                        scalar1=mv[:, ti, 0:1],
                                        scalar2=rstd[:, ti:ti + 1],
                                        op0=OP.subtract, op1=OP.mult)
                for dtg in range(2):
                    tps = psp.tile([128, 4, 128], bf16, tag="tps" + tagsfx)
                    for dq in range(4):
                        dt = dtg * 4 + dq
                        nc.tensor.transpose(tps[:, dq, :],
                                            nrm[:, dt * 128:(dt + 1) * 128],
                                            ident[:])
                    nc.vector.tensor_add(
                        dst[:, dtg * 4:dtg * 4 + 4, dst_col(ti)], tps[:],
                        brep[:, dtg * 4:dtg * 4 + 4, :])

        def rope_apply(dst, ps_ap, n, pool, eng2, bias_col):
            """dst[128, n] bf16 = rope(ps) straight out of PSUM. The rotate
            partner is lane-adjacent (host permuted), so the swap is one
            stream_shuffle. eng2 runs the sin-mul and add (gpsimd offload
            for K; operands of eng2 ops are SBUF-only)."""
            src = ps_ap
            if bias_col is not None:
                tb = pool.tile([128, n], fp32, tag="ropebias")
                nc.vector.tensor_scalar_add(tb[:], ps_ap, bias_col)
                src = tb[:]
            swp = pool.tile([128, n], fp32, tag="ropeswp")
            nc.vector.stream_shuffle(swp[:], src, _SWAP_MASK)
            tcos = pool.tile([128, n], bf16, tag="ropecos")
            nc.vector.tensor_mul(tcos[:], src, cos2[:, 0:n])
            tsin = pool.tile([128, n], bf16, tag="ropesin")
            eng2.tensor_mul(tsin[:], swp[:], sin2[:, 0:n])
            eng2.tensor_add(dst, tcos[:], tsin[:])

        exp_scale = float(1.0 / (S_QK * S_QK * np.sqrt(HEAD_DIM)))

        with tc.tile_pool(name="bc_sbuf", bufs=1) as bcp, \
             tc.tile_pool(name="ptp", bufs=8) as ptp:
            n1T = bcp.tile([128, 8, 1024], fp8)
            Qt = bcp.tile([128, 8, SH], bf16)
            Kt = bcp.tile([128, 8, S], bf16)
            # per head: cols 0:64 = V (x32), cols 64:128 = 1.0 so each attnV
            # matmul also produces the softmax denominator in rows 64:128.
            # The net /32 from V's scale-up is folded into the D epilogue,
            # keeping O_sb ~0.6 magnitude (fp8-friendly).
            Vn = bcp.tile([128, 8, HEADS, 128], fp8)
            nc.vector.memset(Vn[:, :, :, 64:128], 1.0)
            wv_sb = bcp.tile([128, 8, 1024], fp8)

            # ---------- Phase A: x loads + LN1 (st 0-3 first) ----------
            with tc.tile_pool(name="xkv", bufs=4) as xkvp, \
                 tc.tile_pool(name="ln1", bufs=2) as ln1p, \
                 tc.tile_pool(name="ln1ps", bufs=2, space="PSUM") as lnps:
                for st in range(4):
                    eng = (nc.sync, nc.scalar)[st % 2]
                    eng.dma_start(out=x_q[:, st, :], in_=x_d[st])
                kv_tiles = []
                for st in range(4):
                    xkv = xkvp.tile([128, D_MODEL], bf16, tag=f"xkv{st}",
                                    name=f"xkv{st}")
                    eng = (nc.sync, nc.scalar)[st % 2]
                    eng.dma_start(out=xkv[:], in_=xkv_d[st])
                    kv_tiles.append(xkv)
                nc.gpsimd.dma_start(out=cos2[:], in_=cos_d[:])
                nc.gpsimd.dma_start(out=sin2[:], in_=sin_d[:])
                for i in range(2):
                    nc.gpsimd.dma_start(out=betar[i][:], in_=betar_d[i])

                ln_tiles([x_q[:, st, :] for st in range(4)], ln1p, lnps,
                         b0r, n1T, lambda ti: slice(ti * 128, ti * 128 + 128),
                         "1a")

                # wv early on gpsimd (needed by phase V)
                for k in range(8):
                    nc.gpsimd.dma_start(out=wv_sb[:, k, :], in_=wv_d[k])

                ln_tiles([t[:] for t in kv_tiles], ln1p, lnps,
                         b0r, n1T,
                         lambda ti: slice(512 + ti * 128, 512 + ti * 128 + 128),
                         "1a")

            # ---------- Phase B/C ----------
            with tc.tile_pool(name="wstream", bufs=2) as wsp, \
                 tc.tile_pool(name="ropet", bufs=2) as rtp, \
                 tc.tile_pool(name="spsp", bufs=2, space="PSUM") as spsp:

                PTs = {}
                with tc.tile_pool(name="projps", bufs=2, space="PSUM") as qps:
                    for m in range(8):
                        # Q^T[do(m), q]
                        wqt = wsp.tile([128, 8, 128], fp8, tag="wt")
                        nc.sync.dma_start(
                            out=wqt[:],
                            in_=wq_d[m].rearrange("p (a b) -> p a b", b=128))
                        ps = qps.tile([128, SH], fp32, tag="projps")
                        for kp in range(4):
                            nc.tensor.matmul(ps[:], wqt[:, 2 * kp:2 * kp + 2, :],
                                             n1T[:, 2 * kp:2 * kp + 2, 0:SH],
                                             start=(kp == 0), stop=(kp == 3),
                                             perf_mode=DR)
                        rope_apply(Qt[:, m, :], ps[:], SH, rtp, nc.vector,
                                   bias_sb["bq"][:, m:m + 1] if bq_nz else None)

                        # K^T[do(m), k] over all 1024 rows
                        wkt = wsp.tile([128, 8, 128], fp8, tag="wt")
                        nc.sync.dma_start(
                            out=wkt[:],
                            in_=wk_d[m].rearrange("p (a b) -> p a b", b=128))
                        for nh in range(2):
                            ps = qps.tile([128, SH], fp32, tag="projps")
                            for kp in range(4):
                                nc.tensor.matmul(ps[:], wkt[:, 2 * kp:2 * kp + 2, :],
                                                 n1T[:, 2 * kp:2 * kp + 2,
                                                     nh * SH:(nh + 1) * SH],
                                                 start=(kp == 0), stop=(kp == 3),
                                                 perf_mode=DR)
                            rope_apply(Kt[:, m, nh * SH:(nh + 1) * SH], ps[:], SH,
                                       rtp, nc.gpsimd,
                                       bias_sb["bk"][:, m:m + 1] if bk_nz else None)

                        # scores + exp for head pair m (even head rows 0:64,
                        # odd head rows 64:128 on separate PE row groups)
                        PT = [ptp.tile([128, 8, SH], fp8, tag=f"PT{par}",
                                       name=f"PT_{m}_{par}") for par in range(2)]
                        PTs[m] = PT
                        for kb2 in range(4):
                            sps2 = [spsp.tile([128, 2, SH], fp32, tag="sps",
                                              name=f"sps_{m}_{kb2}_{par}")
                                    for par in range(2)]
                            for sub in range(2):
                                kb = 2 * kb2 + sub
                                for par in range(2):
                                    po = par * 64
                                    nc.tensor.matmul(
                                        sps2[par][:, sub, :],
                                        Kt[po:po + 64, m, kb * 128:(kb + 1) * 128],
                                        Qt[po:po + 64, m, :])
                            for par in range(2):
                                if mtriv:
                                    nc.scalar.activation(
                                        PT[par][:, 2 * kb2:2 * kb2 + 2, :],
                                        sps2[par][:], AF.Exp, scale=exp_scale)
                                else:
                                    for sub in range(2):
                                        kb = 2 * kb2 + sub
                                        nc.scalar.activation(
                                            PT[par][:, kb, :],
                                            sps2[par][:, sub, :], AF.Exp,
                                            bias=bias_sb["maskb"][:, kb:kb + 1],
                                            scale=exp_scale)

                    # V natural [s, dv]
                    for st in range(8):
                        for nh in range(2):
                            ps = qps.tile([128, SH], fp32, tag="projps")
                            for kp in range(4):
                                nc.tensor.matmul(
                                    ps[:],
                                    n1T[:, 2 * kp:2 * kp + 2,
                                        st * 128:(st + 1) * 128],
                                    wv_sb[:, 2 * kp:2 * kp + 2,
                                          nh * SH:(nh + 1) * SH],
                                    start=(kp == 0), stop=(kp == 3), perf_mode=DR)
                            src = ps[:]
                            if bv_nz:
                                vtmp = rtp.tile([128, SH], fp32, tag="vtmp")
                                nc.vector.tensor_add(
                                    vtmp[:], ps[:],
                                    bias_sb["bv"][:, nh * SH:(nh + 1) * SH])
                                src = vtmp[:]
                            nc.vector.tensor_copy(
                                out=Vn[:, st, nh * 8:(nh + 1) * 8, 0:64],
                                in_=src.rearrange("p (h d) -> p h d", d=HEAD_DIM))

                    # wo prefetch on the now-idle sync queue
                    for k in range(8):
                        nc.sync.dma_start(out=wo_sb[:, k, :], in_=wo_d[k])
                    if dbg:
                        nc.sync.dma_start(out=dbg_d["d_n1T"][:], in_=n1T[:])
                        nc.sync.dma_start(out=dbg_d["d_qt"][:], in_=Qt[:])
                        nc.sync.dma_start(out=dbg_d["d_kt"][:], in_=Kt[:])
                        nc.sync.dma_start(out=dbg_d["d_vn"][:], in_=Vn[:])
                        nc.sync.dma_start(out=dbg_d["d_pt0"][:], in_=PTs[0][0][:])
                        nc.sync.dma_start(out=dbg_d["d_pt1"][:], in_=PTs[0][1][:])

                # attnV + denominator, normalize. ovdn's banks come from the
                # 2 never-used banks + projps's (spsp stays open so scores
                # banks aren't recycled under the still-draining exp stream).
                with tc.tile_pool(name="ovdn", bufs=4, space="PSUM") as ovp, \
                     tc.tile_pool(name="dnt", bufs=2) as dnp:
                    for m in range(8):
                        PT = PTs[m]
                        pvs = []
                        for h2 in range(2):
                            h = 2 * m + h2
                            pv = ovp.tile([128, SH], fp32, tag="ovdn",
                                          name=f"ov_{m}_{h2}")
                            for kp in range(4):
                                nc.tensor.matmul(
                                    pv[:], Vn[:, 2 * kp:2 * kp + 2, h, :],
                                    PT[h2][:, 2 * kp:2 * kp + 2, :],
                                    start=(kp == 0), stop=(kp == 3),
                                    perf_mode=DR)
                            pvs.append(pv)
                        # rows 0:64 = P@V*32, rows 64:128 = sum(P). Assemble
                        # both heads' denominators into one full-width tile
                        # (reciprocal_approx_fast is broken on partition
                        # slices), then align with partition-shift DMAs.
                        tmp = dnp.tile([128, SH], fp32, tag="dtmp",
                                       name=f"dtmp_{m}")
                        nc.vector.tensor_copy(out=tmp[64:128, :],
                                              in_=pvs[0][64:128, :])
                        dns = dnp.tile([128, SH], fp32, tag="dns",
                                       name=f"dns_{m}")
                        nc.gpsimd.dma_start(out=dns[0:64, :],
                                            in_=tmp[64:128, :])
                        nc.vector.tensor_copy(out=dns[64:128, :],
                                              in_=pvs[1][64:128, :])
                        rr = dnp.tile([128, SH], fp32, tag="rr",
                                      name=f"rr_{m}")
                        nc.vector.reciprocal_approx_fast(out=rr[:], in_=dns[:])
                        nc.vector.tensor_mul(O_sb[0:64, m, :],
                                             pvs[0][0:64, :], rr[0:64, :])
                        rr2 = dnp.tile([128, SH], fp32, tag="rr2",
                                       name=f"rr2_{m}")
                        nc.gpsimd.dma_start(out=rr2[0:64, :],
                                            in_=rr[64:128, :])
                        ot = dnp.tile([128, SH], fp8, tag="ot",
                                      name=f"ot_{m}")
                        nc.vector.tensor_mul(ot[0:64, :],
                                             pvs[1][0:64, :], rr2[0:64, :])
                        nc.gpsimd.dma_start(out=O_sb[64:128, m, :],
                                            in_=ot[0:64, :])

        # table switch to gelu happens here, hidden behind phase D
        nc.scalar.activation(dummy[:], eps_t[:], AF.Gelu)
        if dbg:
            nc.sync.dma_start(out=dbg_d["d_osb"][:], in_=O_sb[:])

        # ---------- Phase D: output projection + residual; LN2; FFN ----------
        with tc.tile_pool(name="ffn", bufs=1) as ffnp:
            n2T = ffnp.tile([128, 8, SH], fp8)
            hT = ffnp.tile([128, 32, SH], fp8)
            w1a = ffnp.tile([128, 16, 8, 128], fp8)
            w2a = ffnp.tile([128, 32, 1024], fp8)
            for j in range(16):
                nc.sync.dma_start(
                    out=w1a[:, j, :, :],
                    in_=w1_d[j].rearrange("p (a b) -> p a b", b=128))
            for j in range(16):
                nc.gpsimd.dma_start(out=w2a[:, j, :], in_=w2_d[j])

            xr = x_q
            if bo_nz:
                xr = ffnp.tile([128, 4, D_MODEL], fp32, tag="xqb")
                for qb in range(4):
                    nc.vector.tensor_add(xr[:, qb, :], x_q[:, qb, :],
                                         bias_sb["bo"][:])

            with tc.tile_pool(name="ops", bufs=2, space="PSUM") as opsp, \
                 tc.tile_pool(name="ln2", bufs=2) as ln2p, \
                 tc.tile_pool(name="ln2ps", bufs=2, space="PSUM") as lnps2:
                for qb in range(4):
                    for nh in range(2):
                        ps = opsp.tile([128, SH], fp32, tag="ops")
                        for kp in range(4):
                            nc.tensor.matmul(
                                ps[:],
                                O_sb[:, 2 * kp:2 * kp + 2, qb * 128:(qb + 1) * 128],
                                wo_sb[:, 2 * kp:2 * kp + 2, nh * SH:(nh + 1) * SH],
                                start=(kp == 0), stop=(kp == 3), perf_mode=DR)
                        sl = slice(nh * SH, (nh + 1) * SH)
                        # 1/S_V undoes O_sb's deliberate x32 carry
                        nc.vector.scalar_tensor_tensor(
                            out=x1[:, qb, sl], in0=ps[:],
                            scalar=1.0 / (S_O * S_V),
                            in1=xr[:, qb, sl], op0=OP.mult, op1=OP.add)
                # LN2 (all 4 qb) -> n2T
                ln_tiles([x1[:, qb, :] for qb in range(4)], ln2p, lnps2,
                         b1r, n2T,
                         lambda ti: slice(ti * 128, ti * 128 + 128), "2")

            if dbg:
                nc.sync.dma_start(out=dbg_d["d_x1"][:], in_=x1[:])
                nc.sync.dma_start(out=dbg_d["d_n2T"][:], in_=n2T[:])
            # second half of w2 streams during FFN1
            for j in range(16, 32):
                nc.gpsimd.dma_start(out=w2a[:, j, :], in_=w2_d[j])

            # FFN1: hT[dff, q] = gelu((w1*S1)^T @ n2^T) via gelu pre-scale
            with tc.tile_pool(name="w1s", bufs=4) as w1p, \
                 tc.tile_pool(name="f1ps", bufs=2, space="PSUM") as f1ps:
                for j in range(32):
                    w1tt = None
                    if j >= 16:
                        w1tt = w1p.tile([128, 8, 128], fp8, tag="w1t")
                        nc.sync.dma_start(
                            out=w1tt[:],
                            in_=w1_d[j].rearrange("p (a b) -> p a b", b=128))
                    ps = f1ps.tile([128, SH], fp32, tag="f1")
                    for kp in range(4):
                        lhs = (w1a[:, j, 2 * kp:2 * kp + 2, :] if j < 16
                               else w1tt[:, 2 * kp:2 * kp + 2, :])
                        nc.tensor.matmul(ps[:], lhs,
                                         n2T[:, 2 * kp:2 * kp + 2, :],
                                         start=(kp == 0), stop=(kp == 3),
                                         perf_mode=DR)
                    if b1_nz:
                        nc.scalar.activation(hT[:, j, :], ps[:], AF.Gelu,
                                             bias=bias_sb["b1"][:, j:j + 1],
                                             scale=1.0 / S_1)
                    else:
                        nc.scalar.activation(hT[:, j, :], ps[:], AF.Gelu,
                                             scale=1.0 / S_1)

            xres = x1
            if b2_nz:
                xres = ffnp.tile([128, 4, D_MODEL], fp32, tag="xres")
                for qb in range(4):
                    nc.vector.tensor_add(xres[:, qb, :], x1[:, qb, :],
                                         bias_sb["b2"][:])

            if dbg:
                nc.sync.dma_start(out=dbg_d["d_hT"][:], in_=hT[:])
            # FFN2: per-qb staggered so epilogues/DMAs overlap later matmuls
            with tc.tile_pool(name="f2ps", bufs=4, space="PSUM") as f2ps, \
                 tc.tile_pool(name="otmp", bufs=4) as otp:
                out_engs = (nc.sync, nc.gpsimd, nc.scalar)
                for qb in range(4):
                    psl = [f2ps.tile([128, SH], fp32, tag="f2",
                                     name=f"f2_{qb}_{nh}") for nh in range(2)]
                    for jp in range(16):
                        for nh in range(2):
                            nc.tensor.matmul(
                                psl[nh][:],
                                hT[:, 2 * jp:2 * jp + 2, qb * 128:(qb + 1) * 128],
                                w2a[:, 2 * jp:2 * jp + 2, nh * SH:(nh + 1) * SH],
                                start=(jp == 0), stop=(jp == 15), perf_mode=DR)
                    for nh in range(2):
                        sl = slice(nh * SH, (nh + 1) * SH)
                        yo = otp.tile([128, SH], fp32, tag="yo")
                        nc.vector.scalar_tensor_tensor(
                            out=yo[:], in0=psl[nh][:], scalar=1.0 / S_2,
                            in1=xres[:, qb, sl], op0=OP.mult, op1=OP.add)
                        if qb < 3:
                            eng = out_engs[(qb * 2 + nh) % 2]
                            eng.dma_start(out=out_d[qb * 128:(qb + 1) * 128, sl],
                                          in_=yo[:])
                        else:
                            # spread the last tiles across 4 queues
                            for q4 in range(2):
                                eng = out_engs[(nh * 2 + q4) % 3]
                                s2 = slice(nh * SH + q4 * 256,
                                           nh * SH + q4 * 256 + 256)
                                eng.dma_start(
                                    out=out_d[qb * 128:(qb + 1) * 128, s2],
                                    in_=yo[:, q4 * 256:q4 * 256 + 256])

    nc.compile()
    return nc


def _lhsT_tile(w, nblocks_in, nblocks_out):
    # w: [in, out] -> [nblocks_out, 128, nblocks_in*128] with
    # result[m][p, k*128+c] = w[k*128+p, m*128+c]
    kin = w.shape[0] // nblocks_in
    return np.ascontiguousarray(
        w.reshape(nblocks_in, kin, nblocks_out, w.shape[1] // nblocks_out)
        .transpose(2, 1, 0, 3)
        .reshape(nblocks_out, kin, -1))


def _fp8(a):
    return np.clip(np.asarray(a, np.float32), -240.0, 240.0).astype(_FP8)


def kernel(src_reps, src_mask, compact_style,
           ada0_w, ada0_b, ada1_w, ada1_b,
           wq, bq, wk, bk, wv, bv, wo, bo,
           w1, b1, w2, b2):
    trace = bool(os.environ.get("KERNEL_TRACE"))
    if trace:
        _install_ntff_shim()
    from concourse.bass_utils import run_bass_kernel_spmd

    src_reps = np.asarray(src_reps, np.float32)
    src_mask = np.asarray(src_mask)
    compact_style = np.asarray(compact_style, np.float32)

    # ---- host prep: adaLN styles ----
    def styles(ada_w, ada_b):
        cs = compact_style
        silu = cs * (1.0 / (1.0 + np.exp(-cs)))
        st = silu @ np.asarray(ada_w, np.float32) + np.asarray(ada_b, np.float32)
        g, be, al = st[:, :D_MODEL], st[:, D_MODEL:2 * D_MODEL], st[:, 2 * D_MODEL:]
        return (1.0 + np.tanh(g) * GAMMA_SCALE), be, al

    m0, be0, al0 = styles(ada0_w, ada0_b)
    m1, be1, al1 = styles(ada1_w, ada1_b)

    # ---- host prep: RoPE head-dim interleave permutation ----
    # new position j within a head holds original dim (j//2) if j even else
    # (j//2 + 32); the rotate partner is then the adjacent lane.
    j = np.arange(HEAD_DIM)
    perm = np.where(j % 2 == 0, j // 2, j // 2 + 32)
    perm_full = (np.arange(D_MODEL) // HEAD_DIM) * HEAD_DIM + \
        np.tile(perm, HEADS)

    # ---- host prep: weights (permute + scale + cast + tile) ----
    # adaLN gamma folds into the input rows of wq/wk/wv/w1 (per batch);
    # beta/gamma is added on-chip to the plain-LN transposed activations.
    wq_f = np.asarray(wq, np.float32)
    wk_f = np.asarray(wk, np.float32)
    wv_f = np.asarray(wv, np.float32)
    w1_f = np.asarray(w1, np.float32)
    wq_b, wk_b, wv_b, w1_b = [], [], [], []
    for b in range(B):
        g0 = m0[b][:, None]
        g1 = m1[b][:, None]
        wq_b.append(_fp8(_lhsT_tile((wq_f * g0)[:, perm_full] * S_QK, 8, 8)))
        wk_b.append(_fp8(_lhsT_tile((wk_f * g0)[:, perm_full] * S_QK, 8, 8)))
        wv_b.append(_fp8((wv_f * g0 * S_V).reshape(8, 128, 1024)))
        w1_b.append(_fp8(_lhsT_tile(w1_f * g1 * S_1, 8, 32)))
    wo_b = [_fp8(((np.asarray(wo, np.float32) * al0[b][None, :]) * S_O)
                 .reshape(8, 128, 1024)) for b in range(B)]
    w2_b = [_fp8(((np.asarray(w2, np.float32) * al1[b][None, :]) * S_2)
                 .reshape(32, 128, 1024)) for b in range(B)]

    flags = (bool(np.all(src_mask)),) + tuple(
        bool(np.any(np.asarray(b) != 0)) for b in (bq, bk, bv, bo, b1, b2))
    if flags not in _graph_cache:
        _graph_cache[flags] = _build_graph(flags)
    nc = _graph_cache[flags]

    # ---- host prep: RoPE tables (permuted rows, sign folded into sin) ----
    inv_freq = 1.0 / (ROPE_BASE **
                      (np.arange(0, HEAD_DIM, 2, dtype=np.float32) / HEAD_DIM))
    # at permuted position j: freq index = j//2, sign = -1 for even j
    fidx = np.arange(HEAD_DIM) // 2
    sign = np.where(np.arange(HEAD_DIM) % 2 == 0, -1.0, 1.0).astype(np.float32)

    def rope_tables(roll):
        pos = np.roll(np.arange(S, dtype=np.float32), -roll)
        ang = pos[None, :] * inv_freq[fidx][:, None]  # [64, S]
        c = np.cos(ang).astype(np.float32)
        s_ = (np.sin(ang) * sign[:, None]).astype(np.float32)
        return (np.ascontiguousarray(np.concatenate([c, c], 0)).astype(_BF16),
                np.ascontiguousarray(np.concatenate([s_, s_], 0)).astype(_BF16))

    tables = [rope_tables(0), rope_tables(SH)]

    in_maps = []
    for c in range(N_CORES):
        b, h = c // 2, c % 2
        x_c = np.roll(src_reps[b], -h * SH, axis=0)
        # beta/gamma replicated along the inner 128 columns:
        # betar[i][p, k, :] = (beta_i/gamma_i)[k*128+p]
        bp = np.stack([be0[b] / m0[b], be1[b] / m1[b]])  # [2, 1024]
        betar = np.broadcast_to(
            bp.reshape(2, 8, 128).transpose(0, 2, 1)[:, :, :, None],
            (2, 128, 8, 128))
        im = {
            "x": np.ascontiguousarray(x_c[0:512].reshape(4, 128, D_MODEL)),
            "xkv": np.ascontiguousarray(
                x_c[512:1024].reshape(4, 128, D_MODEL)).astype(_BF16),
            "wq": wq_b[b], "wk": wk_b[b], "wv": wv_b[b], "wo": wo_b[b],
            "w1": w1_b[b], "w2": w2_b[b],
            "cos2": tables[h][0], "sin2": tables[h][1],
            "betar": np.ascontiguousarray(np.clip(betar, -240, 240).astype(_FP8)),
        }
        if not flags[0]:
            mb = np.where(np.roll(src_mask[b], -h * SH), 0.0, -60.0)
            im["maskb"] = np.ascontiguousarray(
                mb.reshape(8, 128).T.astype(np.float32))
        if flags[1]:
            im["bq"] = np.ascontiguousarray(
                (np.asarray(bq, np.float32) * S_QK)[perm_full]
                .reshape(8, 128).T)
        if flags[2]:
            im["bk"] = np.ascontiguousarray(
                (np.asarray(bk, np.float32) * S_QK)[perm_full]
                .reshape(8, 128).T)
        if flags[3]:
            im["bv"] = np.asarray(bv, np.float32) * S_V
        if flags[4]:
            im["bo"] = np.asarray(bo, np.float32) * al0[b]
        if flags[5]:
            im["b1"] = np.ascontiguousarray(
                (np.asarray(b1, np.float32) * S_1).reshape(32, 128).T)
        if flags[6]:
            im["b2"] = np.asarray(b2, np.float32) * al1[b]
        in_maps.append(im)

    res = run_bass_kernel_spmd(nc, in_maps, core_ids=list(range(N_CORES)),
                               trace=trace)
    kernel.last_result = res

    out = np.empty((B, S, D_MODEL), np.float32)
    for c in range(N_CORES):
        b, h = c // 2, c % 2
        out[b, h * SH:(h + 1) * SH, :] = res.results[c]["out"]
    return out
